# revision 12
# baseline (speedup 1.0000x reference)
"""MoE transformer block (attention + top-2 routed 8-expert FFN) on 8 TRN2
NeuronCores.

Sharding: data-parallel attention (1 image of 196 tokens per core) +
expert-parallel MoE (1 expert per core). Token dispatch/combine via
AllGather + dma_gather, with slot ranks computed on-device by a cumsum
scan over the routing mask.

Self-contained: hardcodes all shapes; imports only concourse (already on
PYTHONPATH in the runtime image).
"""
import os
import sys
for _p in ("/opt/trn_rl_repo", "/root/.axon_site/_ro/trn_rl_repo"):
    if _p not in sys.path:
        sys.path.append(_p)

import numpy as np
import concourse.bass as bass
import concourse.bacc as bacc
import concourse.mybir as mybir
import concourse.tile as tile
from concourse import masks
from concourse import bass_utils

F32 = mybir.dt.float32
F32R = mybir.dt.float32r
I16 = mybir.dt.int16
I32 = mybir.dt.int32
U32 = mybir.dt.uint32
AF = mybir.ActivationFunctionType
OP = mybir.AluOpType

NCORES = 8
B, N, D = 8, 196, 768
HEADS, HD = 12, 64
E, HID = 8, 3072
T = B * N                  # 1568
C = 512                    # expert capacity (max observed count is 424)
KT = D // 128              # 6 feature k-tiles
HM = HID // 128            # 24 hidden tiles
CM = C // 128              # 4 capacity tiles
LN_EPS = 1e-5
ROWB = 832                 # AG row: 768 xn + 8 logits + pad to 256B multiple
TW = 13                    # ceil(T/128) wrapped token tiles
TWPAD = TW * 128           # 1664
NCH = [128, N - 128]       # valid partitions per token chunk (128 + 68)


def _repl16(nc, dst, src16):
    """Replicate a [16, f] tile across all 128 partitions of dst."""
    nc.vector.memset(dst[:], 0)
    nc.vector.tensor_copy(dst[0:16, :], src16[:])
    nc.vector.stream_shuffle(dst[0:32, :], dst[0:32, :],
                             mask=[i % 16 for i in range(32)])
    nc.vector.tensor_copy(dst[32:64, :], dst[0:32, :])
    nc.vector.tensor_copy(dst[64:128, :], dst[0:64, :])


def build_block(tc, outs, ins):
    nc = tc.nc
    out = outs["out"]          # [196, 768] per-core output
    dbg = outs.get("dbg", {})  # optional {name: AP} debug outputs

    sb = tc.alloc_tile_pool(name="sb", bufs=1)      # small persistents
    sbw = tc.alloc_tile_pool(name="sbw", bufs=3)    # loop workspace
    dram = tc.alloc_tile_pool(name="dram", bufs=1, space="DRAM")

    ident = sb.tile([128, 128], F32)
    masks.make_identity(nc, ident[:])
    ones_col = sb.tile([128, 1], F32)
    nc.vector.memset(ones_col[:], 1.0)
    ones8 = sb.tile([8, 1], F32)
    nc.vector.memset(ones8[:], 1.0)
    eps_sb = sb.tile([1, 1], F32)
    nc.vector.memset(eps_sb[:], LN_EPS)

    cvec = {}
    for nm, w in (("qkv_b", 18), ("proj_b", KT), ("ln1_g", KT), ("ln1_b", KT),
                  ("ln2_g", KT), ("ln2_b", KT), ("b1", HM)):
        t = sb.tile([128, w], F32, tag=nm)
        nc.sync.dma_start(t[:], ins[nm][:])
        cvec[nm] = t
    b2_row = sb.tile([1, D], F32, tag="b2r")
    nc.sync.dma_start(b2_row[:], ins["b2"][:])
    b2_b = sb.tile([128, D], F32, tag="b2b")
    nc.gpsimd.partition_broadcast(b2_b[:], b2_row[:])

    tok0_sb = sb.tile([1, 1], I32, tag="tok0")
    nc.sync.dma_start(tok0_sb[:], ins["tok0"][:])
    eoh_sb = sb.tile([128, 1], F32, tag="eoh")
    nc.sync.dma_start(eoh_sb[:], ins["eoh"][:])

    # medium-lived persistents
    x1T = sb.tile([128, KT, N], F32, tag="x1T")
    x1tok = sb.tile([128, 2, D], F32, tag="x1tok")

    # ---- feature-major LayerNorm via matmul-ones stats -------------------
    def layer_norm_fm(src, g_t, b_t, dst, psln, wk):
        s1 = psln.tile([1, N], F32, tag="lnp1")
        for k in range(KT):
            nc.tensor.matmul(s1[:], ones_col[:], src[:, k, :],
                             start=(k == 0), stop=(k == KT - 1))
        s2 = psln.tile([1, N], F32, tag="lnp2")
        for k in range(KT):
            sq = wk.tile([128, N], F32, tag="ln_sq")
            nc.scalar.activation(sq[:], src[:, k, :], AF.Square)
            nc.tensor.matmul(s2[:], ones_col[:], sq[:],
                             start=(k == 0), stop=(k == KT - 1))
        mu = wk.tile([1, N], F32, tag="ln_mu")
        nc.vector.tensor_scalar_mul(mu[:], s1[:], 1.0 / D)
        nmu2 = wk.tile([1, N], F32, tag="ln_nmu2")
        nc.vector.scalar_tensor_tensor(nmu2[:], mu[:], -1.0, mu[:],
                                       op0=OP.mult, op1=OP.mult)
        var = wk.tile([1, N], F32, tag="ln_var")
        nc.vector.scalar_tensor_tensor(var[:], s2[:], 1.0 / D, nmu2[:],
                                       op0=OP.mult, op1=OP.add)
        sd = wk.tile([1, N], F32, tag="ln_sd")
        nc.scalar.activation(sd[:], var[:], AF.Sqrt, bias=eps_sb[:])
        rstd = wk.tile([1, N], F32, tag="ln_rstd")
        nc.vector.reciprocal(rstd[:], sd[:])
        mu_b = wk.tile([128, N], F32, tag="ln_mub")
        nc.gpsimd.partition_broadcast(mu_b[:], mu[:])
        rstd_b = wk.tile([128, N], F32, tag="ln_rstdb")
        nc.gpsimd.partition_broadcast(rstd_b[:], rstd[:])
        for k in range(KT):
            tmp = wk.tile([128, N], F32, tag="ln_tmp")
            nc.vector.tensor_tensor(tmp[:], src[:, k, :], mu_b[:],
                                    op=OP.subtract)
            nc.vector.tensor_tensor(tmp[:], tmp[:], rstd_b[:], op=OP.mult)
            nc.any.tensor_scalar(dst[:, k, :], tmp[:], g_t[:, k:k + 1],
                                 b_t[:, k:k + 1], op0=OP.mult, op1=OP.add)

    # =====================================================================
    # Phase A: attention (fp32, feature-major)
    # =====================================================================
    with (
        tc.tile_pool(name="p_attn", bufs=1) as pa,
        tc.tile_pool(name="p_qkvw", bufs=1) as pqw,
    ):
        xT_sb = pa.tile([128, KT, N], F32, tag="xT")
        projw_sb = pa.tile([128, KT, D], F32, tag="projw")
        qkvw_sb = pqw.tile([128, KT, 2304], F32, tag="qkvw")
        for k in range(KT):
            sl = slice(128 * k, 128 * (k + 1))
            nc.sync.dma_start(xT_sb[:, k, :], ins["xT"][sl, :])
            nc.sync.dma_start(projw_sb[:, k, :], ins["proj_w"][sl, :])
            nc.sync.dma_start(qkvw_sb[:, k, :], ins["qkv_w"][sl, :])

        xn1 = pa.tile([128, KT, N], F32, tag="xn1")
        with (
            tc.tile_pool(name="psln", bufs=1, space="PSUM") as psln,
            tc.tile_pool(name="wkln1", bufs=2) as wkln1,
        ):
            layer_norm_fm(xT_sb, cvec["ln1_g"], cvec["ln1_b"], xn1, psln,
                          wkln1)

        qkvT = pa.tile([128, 18, N], F32, tag="qkvT")
        with tc.tile_pool(name="psqkv", bufs=3, space="PSUM") as psqkv:
            for m in range(18):
                ps = psqkv.tile([128, N], F32, tag="qkvps")
                for k in range(KT):
                    nc.tensor.matmul(
                        ps[:], qkvw_sb[:, k, 128 * m:128 * (m + 1)],
                        xn1[:, k, :], start=(k == 0), stop=(k == KT - 1))
                nc.any.tensor_scalar(qkvT[:, m, :], ps[:],
                                     cvec["qkv_b"][:, m:m + 1], None,
                                     op0=OP.add)

        def head_slice(base, h):
            return qkvT[64 * (h % 2):64 * (h % 2) + 64, base + h // 2, :]

        attn_out = pa.tile([128, 2, D], F32, tag="attn_out")  # token-major
        with (
            tc.tile_pool(name="pss", bufs=2, space="PSUM") as pss,
            tc.tile_pool(name="pst", bufs=3, space="PSUM") as pst,
            tc.tile_pool(name="pso", bufs=2, space="PSUM") as pso,
            tc.tile_pool(name="wka", bufs=3) as wka,
        ):
            for h in range(HEADS):
                qT, kT, vT = (head_slice(b, h) for b in (0, 6, 12))
                b0 = HD * (h % 2)
                v_tok = wka.tile([128, 2, HD], F32, tag="v_tok")
                for i, sz in enumerate(NCH):
                    pv = pst.tile([128, 128], F32, tag="ptt")
                    nc.tensor.transpose(pv[:sz, :HD],
                                        vT[:, 128 * i:128 * i + sz],
                                        ident[b0:b0 + HD, b0:b0 + HD])
                    nc.any.tensor_copy(v_tok[:sz, i, :], pv[:sz, :HD])
                attnT = wka.tile([128, 2, N], F32, tag="attnT")
                rcp = wka.tile([128, 2], F32, tag="rcp")
                for i, sz in enumerate(NCH):
                    ps = pss.tile([128, N], F32, tag="sps")
                    nc.tensor.matmul(ps[:sz, :], qT[:, 128 * i:128 * i + sz],
                                     kT[:], start=True, stop=True)
                    m0 = wka.tile([128, 1], F32, tag="sm_m0")
                    nc.vector.tensor_reduce(m0[:sz], ps[:sz, :],
                                            axis=mybir.AxisListType.X,
                                            op=OP.max)
                    m0n = wka.tile([128, 1], F32, tag="sm_m0n")
                    nc.vector.tensor_scalar_mul(m0n[:sz], m0[:sz], -0.125)
                    ex = wka.tile([128, N], F32, tag="sm_ex")
                    ssum = wka.tile([128, 1], F32, tag="sm_sum")
                    nc.scalar.activation(ex[:sz, :], ps[:sz, :], AF.Exp,
                                         bias=m0n[:sz], scale=0.125,
                                         accum_out=ssum[:sz])
                    nc.vector.reciprocal(rcp[:sz, i:i + 1], ssum[:sz])
                    for j, szj in enumerate(NCH):
                        pt = pst.tile([128, 128], F32, tag="ptt")
                        nc.tensor.transpose(pt[:szj, :sz],
                                            ex[:sz, 128 * j:128 * j + szj],
                                            ident[0:sz, 0:sz])
                        nc.any.tensor_copy(
                            attnT[:szj, j, 128 * i:128 * i + sz],
                            pt[:szj, :sz])
                for i, sz in enumerate(NCH):
                    po = pso.tile([128, HD], F32, tag="pso")
                    for j, szj in enumerate(NCH):
                        nc.tensor.matmul(po[:sz, :],
                                         attnT[:szj, j, 128 * i:128 * i + sz],
                                         v_tok[:szj, j, :],
                                         start=(j == 0), stop=(j == 1))
                    nc.any.tensor_scalar(
                        attn_out[:sz, i, 64 * h:64 * (h + 1)], po[:sz, :],
                        rcp[:sz, i:i + 1], None, op0=OP.mult)

        aoT = pa.tile([128, KT, N], F32, tag="aoT")
        with (
            tc.tile_pool(name="psat", bufs=3, space="PSUM") as psat,
            tc.tile_pool(name="psp", bufs=3, space="PSUM") as psp,
        ):
            for ft in range(KT):
                pt = psat.tile([128, N], F32, tag="psatt")
                for i, sz in enumerate(NCH):
                    nc.tensor.transpose(
                        pt[:, 128 * i:128 * i + sz],
                        attn_out[:sz, i, 128 * ft:128 * (ft + 1)],
                        ident[0:sz, 0:sz])
                nc.any.tensor_copy(aoT[:, ft, :], pt[:])
            for ft in range(KT):
                ps = psp.tile([128, N], F32, tag="pspp")
                for k in range(KT):
                    nc.tensor.matmul(ps[:],
                                     projw_sb[:, k, 128 * ft:128 * (ft + 1)],
                                     aoT[:, k, :], start=(k == 0),
                                     stop=(k == KT - 1))
                nc.vector.scalar_tensor_tensor(
                    x1T[:, ft, :], ps[:], cvec["proj_b"][:, ft:ft + 1],
                    xT_sb[:, ft, :], op0=OP.add, op1=OP.add)

    # =====================================================================
    # Phase B: LN2, bounce assembly, AllGather #1
    # =====================================================================
    ag_in = dram.tile([N, ROWB], F32)
    ag_out = dram.tile([T, ROWB], F32)
    with tc.tile_pool(name="p_gate", bufs=1) as pg:
        gatew_sb = pg.tile([128, KT, 8], F32, tag="gatew")
        for k in range(KT):
            nc.sync.dma_start(gatew_sb[:, k, :],
                              ins["gate_w"][128 * k:128 * (k + 1), :])
        xnT = pg.tile([128, KT, N], F32, tag="xnT")
        with (
            tc.tile_pool(name="psln2", bufs=1, space="PSUM") as psln2,
            tc.tile_pool(name="wkln2", bufs=2) as wkln2,
        ):
            layer_norm_fm(x1T, cvec["ln2_g"], cvec["ln2_b"], xnT, psln2,
                          wkln2)

        bounce = pg.tile([128, 2, ROWB], F32, tag="bounce")
        nc.vector.memset(bounce[:, :, D + 8:ROWB], 0.0)
        with tc.tile_pool(name="psb", bufs=2, space="PSUM") as psb:
            for i, sz in enumerate(NCH):
                pt = psb.tile([128, D], F32, tag="psbt")
                for ft in range(KT):
                    nc.tensor.transpose(pt[:sz, 128 * ft:128 * (ft + 1)],
                                        xnT[:, ft, 128 * i:128 * i + sz],
                                        ident[:])
                nc.any.tensor_copy(bounce[:sz, i, 0:D], pt[:sz, :])
                pl = psb.tile([128, 8], F32, tag="psbl")
                for k in range(KT):
                    nc.tensor.matmul(pl[:sz, :],
                                     xnT[:, k, 128 * i:128 * i + sz],
                                     gatew_sb[:, k, :], start=(k == 0),
                                     stop=(k == KT - 1))
                nc.any.tensor_copy(bounce[:sz, i, D:D + 8], pl[:sz, :])
        for i, sz in enumerate(NCH):
            nc.sync.dma_start(ag_in[128 * i:128 * i + sz, :],
                              bounce[:sz, i, :])
        if "xnT" in dbg:
            for k in range(KT):
                nc.sync.dma_start(dbg["xnT"][128 * k:128 * (k + 1), :],
                                  xnT[:, k, :])
    nc.gpsimd.collective_compute(
        "AllGather", OP.bypass, ins=[ag_in.opt()], outs=[ag_out.opt()],
        replica_groups=[list(range(NCORES))])

    # =====================================================================
    # Phase C: routing (replicated on every core)
    # =====================================================================
    lg = sb.tile([128, TW, 8], F32, tag="lg")
    nc.vector.memset(lg[:], 0.0)
    nc.sync.dma_start(
        lg[:, 0:12, :],
        ag_out[0:1536, D:D + 8].rearrange("(i p) c -> p i c", p=128))
    nc.sync.dma_start(lg[0:32, 12, :], ag_out[1536:T, D:D + 8])

    gates = sb.tile([128, TW, 8], F32, tag="gates")
    g0wr = sb.tile([128, TW], F32, tag="g0wr")
    for i in range(TW):
        lgi = lg[:, i, :]
        m0 = sbw.tile([128, 1], F32, tag="rt_m0")
        nc.vector.tensor_reduce(m0[:], lgi, axis=mybir.AxisListType.X,
                                op=OP.max)
        m0n = sbw.tile([128, 1], F32, tag="rt_m0n")
        nc.vector.tensor_scalar_mul(m0n[:], m0[:], -1.0)
        mk0 = sbw.tile([128, 8], F32, tag="rt_mk0")
        nc.vector.tensor_scalar(mk0[:], lgi, m0[:], None, op0=OP.is_ge)
        msk = sbw.tile([128, 8], F32, tag="rt_msk")
        nc.vector.scalar_tensor_tensor(msk[:], mk0[:], -1e30, lgi,
                                       op0=OP.mult, op1=OP.add)
        m1 = sbw.tile([128, 1], F32, tag="rt_m1")
        nc.vector.tensor_reduce(m1[:], msk[:], axis=mybir.AxisListType.X,
                                op=OP.max)
        ex = sbw.tile([128, 8], F32, tag="rt_ex")
        nc.scalar.activation(ex[:], lgi, AF.Exp, bias=m0n[:])
        e1 = sbw.tile([128, 1], F32, tag="rt_e1")
        nc.scalar.activation(e1[:], m1[:], AF.Exp, bias=m0n[:])
        den = sbw.tile([128, 1], F32, tag="rt_den")
        nc.vector.tensor_scalar_add(den[:], e1[:], 1.0)
        nc.vector.reciprocal(g0wr[:, i:i + 1], den[:])
        mt = sbw.tile([128, 8], F32, tag="rt_mt")
        nc.vector.tensor_scalar(mt[:], lgi, m1[:], None, op0=OP.is_ge)
        gu = sbw.tile([128, 8], F32, tag="rt_gu")
        nc.vector.tensor_tensor(gu[:], ex[:], mt[:], op=OP.mult)
        nc.vector.tensor_scalar(gates[:, i, :], gu[:], g0wr[:, i:i + 1],
                                None, op0=OP.mult)

    p_rt = tc.alloc_tile_pool(name="p_rt", bufs=1)
    g8_t = p_rt.tile([8, TWPAD], F32, tag="g8")
    mask8_t = p_rt.tile([8, TWPAD], F32, tag="mask8")
    rank8_t = p_rt.tile([8, TWPAD], F32, tag="rank8")
    slotA_t = p_rt.tile([8, TWPAD], F32, tag="slotA")
    sel0_t = p_rt.tile([8, TWPAD], F32, tag="sel0")
    sel1_t = p_rt.tile([8, TWPAD], F32, tag="sel1")
    s0A_t = p_rt.tile([8, TWPAD], F32, tag="s0A")
    s1A_t = p_rt.tile([8, TWPAD], F32, tag="s1A")
    s0G_t = p_rt.tile([8, TWPAD], F32, tag="s0G")
    mrow_t = p_rt.tile([1, TWPAD], F32, tag="mrow")
    g8 = g8_t[:]
    mask8 = mask8_t[:]
    rank8 = rank8_t[:]
    slotA = slotA_t[:]
    sel0 = sel0_t[:]
    sel1 = sel1_t[:]
    s0A = s0A_t[:]
    s1A = s1A_t[:]
    s0G = s0G_t[:]
    mrow = mrow_t[:]
    with tc.tile_pool(name="psg", bufs=3, space="PSUM") as psg:
        for i in range(TW):
            pt = psg.tile([8, 128], F32, tag="psgt")
            nc.tensor.transpose(pt[:], gates[:, i, :], ident[:])
            nc.any.tensor_copy(g8[:, 128 * i:128 * (i + 1)], pt[:])

    nc.vector.tensor_scalar(mask8, g8, 0.0, None, op0=OP.is_gt)
    nc.vector.tensor_tensor_scan(rank8, mask8, mask8, initial=0.0,
                                 op0=OP.add, op1=OP.bypass)
    eoff_i = sb.tile([8, 1], I32, tag="eoff_i")
    nc.gpsimd.iota(eoff_i[:], pattern=[[1, 1]], base=-1, channel_multiplier=C)
    eoff = sb.tile([8, 1], F32, tag="eoff")
    nc.vector.tensor_copy(eoff[:], eoff_i[:])
    nc.vector.tensor_scalar(slotA, rank8, eoff[:], None, op0=OP.add)
    nc.vector.tensor_scalar(sel0, g8, 0.5, None, op0=OP.is_ge)
    nc.vector.tensor_tensor(sel1, mask8, sel0, op=OP.subtract)
    nc.vector.tensor_tensor(s0A, sel0, slotA, op=OP.mult)
    nc.vector.tensor_tensor(s1A, sel1, slotA, op=OP.mult)
    nc.vector.tensor_tensor(s0G, sel0, g8, op=OP.mult)

    # my-token window [tok0, tok0+196) via dynamic rhs slice
    treg = nc.tensor.alloc_register("tok0reg")
    nc.tensor.reg_load(treg, tok0_sb[0:1, 0:1])
    toff = nc.tensor.snap(treg, donate=True, min_val=0,
                          max_val=N * (NCORES - 1))

    c0my16 = sb.tile([1, N], I16, tag="c0my16")
    c1my16 = sb.tile([1, N], I16, tag="c1my16")
    g0my = sb.tile([1, N], F32, tag="g0my")
    with tc.tile_pool(name="psc", bufs=3, space="PSUM") as psc:
        for srcv, base, dst in ((s0A, 0, c0my16), (s1A, 0, c1my16),
                                (s0G, 0, g0my)):
            p = psc.tile([1, N], F32, tag="pscc")
            nc.tensor.matmul(p[:], ones_col[base:base + 8, 0:1],
                             srcv[:, bass.ds(toff, N)],
                             start=True, stop=True)
            nc.any.tensor_copy(dst[:], p[:])

    # combine idx: [c0 | pad | c1 | pad] -> dram[512] -> wrapped16 -> 128p
    c01 = sb.tile([1, 512], I16, tag="c01")
    nc.vector.memset(c01[:], 0)
    nc.vector.tensor_copy(c01[:, 0:N], c0my16[:])
    nc.vector.tensor_copy(c01[:, 256:256 + N], c1my16[:])
    cidx_d = dram.tile([1, 512], I16)
    nc.sync.dma_start(cidx_d[:], c01[:])
    cidx16 = sb.tile([16, 32], I16, tag="cidx16")
    nc.sync.dma_start(cidx16[:], cidx_d[:].rearrange("1 (f p) -> p f", p=16))
    cidx128 = sb.tile([128, 32], I16, tag="cidx128")
    _repl16(nc, cidx128, cidx16)

    # g0 window -> wrapped [128, 2]
    g0_d = dram.tile([1, N], F32)
    nc.sync.dma_start(g0_d[:], g0my[:])
    g0w2 = sb.tile([128, 2], F32, tag="g0w2")
    nc.vector.memset(g0w2[:], 0.0)
    nc.sync.dma_start(g0w2[:, 0:1], g0_d[0:1, 0:128].rearrange("1 p -> p 1"))
    nc.sync.dma_start(g0w2[0:N - 128, 1:2],
                      g0_d[0:1, 128:N].rearrange("1 p -> p 1"))

    # dispatch idx: my expert row of mask8 -> compacted token list
    with tc.tile_pool(name="psm", bufs=2, space="PSUM") as psm:
        for c0, cw in ((0, 512), (512, 512), (1024, 512), (1536, 128)):
            p = psm.tile([1, cw], F32, tag="psmm")
            nc.tensor.matmul(p[:], eoh_sb[0:8, 0:1], mask8[:, c0:c0 + cw],
                             start=True, stop=True)
            nc.any.tensor_copy(mrow[0:1, c0:c0 + cw], p[:])
    mrow_d = dram.tile([1, TWPAD], F32)
    nc.sync.dma_start(mrow_d[:], mrow)
    m16 = sb.tile([16, T // 16], F32, tag="m16")
    nc.sync.dma_start(m16[:],
                      mrow_d[0:1, 0:T].rearrange("1 (f p) -> p f", p=16))
    iot = sb.tile([16, T // 16], I32, tag="iot")
    nc.gpsimd.iota(iot[:], pattern=[[16, T // 16]], base=0,
                   channel_multiplier=1)
    iotf = sb.tile([16, T // 16], F32, tag="iotf")
    nc.vector.tensor_copy(iotf[:], iot[:])
    cand = sb.tile([16, T // 16], F32, tag="cand")
    nc.vector.tensor_scalar_add(cand[:], iotf[:], 1.0)
    nc.vector.tensor_tensor(cand[:], cand[:], m16[:], op=OP.mult)
    nc.vector.tensor_scalar_add(cand[:], cand[:], -1.0)
    nfound = sb.tile([1, 1], U32, tag="nfound")
    comp = sb.tile([16, C // 16], F32, tag="comp")
    nc.gpsimd.sparse_gather(comp[:], cand[:], num_found=nfound[:])
    nc.vector.tensor_scalar(comp[:], comp[:], float(T - 1), 0.0,
                            op0=OP.min, op1=OP.max)
    didx16 = sb.tile([16, C // 16], I16, tag="didx16")
    nc.vector.tensor_copy(didx16[:], comp[:])
    didx128 = sb.tile([128, C // 16], I16, tag="didx128")
    _repl16(nc, didx128, didx16)

    if "rank8" in dbg:
        nc.sync.dma_start(dbg["rank8"][:], rank8)
    p_rt.release()

    # =====================================================================
    # Phase D: dispatch gather, expert FFN, AllGather #2
    # =====================================================================
    y_d = dram.tile([C, D], F32)
    p_hT = tc.alloc_tile_pool(name="p_hT", bufs=1)
    with tc.tile_pool(name="p_w1", bufs=1) as pw1:
        w1_sb = pw1.tile([128, KT, HID], F32R, tag="w1")
        for k in range(KT):
            nc.sync.dma_start(w1_sb[:, k, :],
                              ins["w1"][128 * k:128 * (k + 1), :]
                              .bitcast(F32R))
        XgT = pw1.tile([128, KT, C], F32R, tag="XgT")
        with tc.tile_pool(name="p_xg", bufs=1) as pxg:
            Xg = pxg.tile([128, CM, D], F32, tag="Xg")
            nc.gpsimd.dma_gather(Xg[:], ag_out[:, 0:D], didx128[:],
                                 num_idxs=C, num_idxs_reg=C, elem_size=D,
                                 elem_step=ROWB)
            with tc.tile_pool(name="psx", bufs=3, space="PSUM") as psx:
                for ct in range(CM):
                    for ft in range(KT):
                        pt = psx.tile([128, 128], F32, tag="psxt")
                        nc.tensor.transpose(
                            pt[:], Xg[:, ct, 128 * ft:128 * (ft + 1)],
                            ident[:])
                        nc.any.tensor_copy(
                            XgT[:, ft, 128 * ct:128 * (ct + 1)], pt[:])

        hT = p_hT.tile([128, HM, C], F32R, tag="hT")
        with tc.tile_pool(name="ps1", bufs=3, space="PSUM") as ps1:
            for m in range(HM):
                ph = ps1.tile([128, C], F32, tag="ps1t")
                for k in range(KT):
                    nc.tensor.matmul(ph[:],
                                     w1_sb[:, k, 128 * m:128 * (m + 1)],
                                     XgT[:, k, :], start=(k == 0),
                                     stop=(k == KT - 1))
                nc.scalar.activation(hT[:, m, :], ph[:], AF.Gelu,
                                     bias=cvec["b1"][:, m:m + 1])

    with (
        tc.tile_pool(name="p_y", bufs=1) as py,
        tc.tile_pool(name="ps4", bufs=1, space="PSUM") as ps4,
        tc.tile_pool(name="w2p", bufs=3) as w2p,
    ):
        y_sb = py.tile([128, CM, D], F32, tag="y_sb")
        yps = {}
        for mc in range(CM):
            for nb in range(2):
                ypst = ps4.tile([128, 384], F32, tag=f"y{mc}{nb}")
                yps[(mc, nb)] = ypst
        for k in range(HM):
            w2t = w2p.tile([128, D], F32R, tag="w2t")
            nc.sync.dma_start(w2t[:],
                              ins["w2"][128 * k:128 * (k + 1), :]
                              .bitcast(F32R))
            for mc in range(CM):
                for nb in range(2):
                    nc.tensor.matmul(
                        yps[(mc, nb)][:],
                        hT[:, k, 128 * mc:128 * (mc + 1)],
                        w2t[:, 384 * nb:384 * (nb + 1)],
                        start=(k == 0), stop=(k == HM - 1))
        for mc in range(CM):
            for nb in range(2):
                nc.any.tensor_copy(y_sb[:, mc, 384 * nb:384 * (nb + 1)],
                                   yps[(mc, nb)][:])
        nc.sync.dma_start(y_d[:].rearrange("(i p) d -> p i d", p=128),
                          y_sb[:])
        if "ysb" in dbg:
            nc.sync.dma_start(
                dbg["ysb"][:].rearrange("(i p) d -> p i d", p=128), y_sb[:])

    p_hT.release()
    yall = dram.tile([NCORES * C, D], F32)
    nc.gpsimd.collective_compute(
        "AllGather", OP.bypass, ins=[y_d.opt()], outs=[yall.opt()],
        replica_groups=[list(range(NCORES))])

    # x1 token-major + b2 (independent of expert compute; overlaps)
    with tc.tile_pool(name="psr", bufs=2, space="PSUM") as psr:
        for i, sz in enumerate(NCH):
            pt = psr.tile([128, D], F32, tag="psrt")
            for ft in range(KT):
                nc.tensor.transpose(pt[:sz, 128 * ft:128 * (ft + 1)],
                                    x1T[:, ft, 128 * i:128 * i + sz],
                                    ident[:])
            nc.vector.tensor_tensor(x1tok[:sz, i, :], pt[:sz, :],
                                    b2_b[:sz, :], op=OP.add)

    # =====================================================================
    # Phase E: combine gather + weighted sum + output
    # =====================================================================
    with tc.tile_pool(name="p_e", bufs=1) as pe:
        g_tok = pe.tile([128, 4, D], F32, tag="g_tok")
        nc.gpsimd.dma_gather(g_tok[:], yall[:], cidx128[:],
                             num_idxs=512, num_idxs_reg=512, elem_size=D)
        for i, sz in enumerate(NCH):
            dtile = sbw.tile([128, D], F32, tag="cmb_d")
            nc.vector.tensor_tensor(dtile[:sz, :], g_tok[:sz, i, :],
                                    g_tok[:sz, i + 2, :], op=OP.subtract)
            t2 = sbw.tile([128, D], F32, tag="cmb_t2")
            nc.vector.scalar_tensor_tensor(t2[:sz, :], dtile[:sz, :],
                                           g0w2[:sz, i:i + 1],
                                           g_tok[:sz, i + 2, :],
                                           op0=OP.mult, op1=OP.add)
            ocmb = sbw.tile([128, D], F32, tag="cmb_o")
            nc.vector.tensor_tensor(ocmb[:sz, :], t2[:sz, :],
                                    x1tok[:sz, i, :], op=OP.add)
            nc.sync.dma_start(out[128 * i:128 * i + sz, :], ocmb[:sz, :])

    # remaining debug taps
    if "gates" in dbg:
        for i in range(TW):
            nc.sync.dma_start(dbg["gates"][0:128, 8 * i:8 * (i + 1)],
                              gates[:, i, :])
    if "cidx" in dbg:
        nc.sync.dma_start(dbg["cidx"][:], cidx128[0:16, :])
    if "didx" in dbg:
        nc.sync.dma_start(dbg["didx"][:], didx128[0:16, :])
    if "g0w2" in dbg:
        nc.sync.dma_start(dbg["g0w2"][:], g0w2[:])
    if "x1T" in dbg:
        for k in range(KT):
            nc.sync.dma_start(dbg["x1T"][128 * k:128 * (k + 1), :],
                              x1T[:, k, :])

    sbw.release()
    sb.release()
    dram.release()


# ---------------------------------------------------------------------------
# host side
# ---------------------------------------------------------------------------

DBG_SPECS = {
    "xnT": (D, N), "gates": (128, TW * 8), "rank8": (8, TWPAD),
    "cidx": (16, 32), "didx": (16, C // 16), "g0w2": (128, 2),
    "ysb": (C, D), "x1T": (D, N),
}

_PROGRAM_CACHE = {}


def get_program(debug=False):
    key = bool(debug)
    if key in _PROGRAM_CACHE:
        return _PROGRAM_CACHE[key]
    nc = bacc.Bacc("TRN2", target_bir_lowering=False, debug=False,
                   enable_asserts=True, num_devices=NCORES)
    ins = {}

    def din(name, shape, dtype=F32):
        ins[name] = nc.dram_tensor(name, list(shape), dtype,
                                   kind="ExternalInput").ap()

    din("xT", (D, N))
    din("qkv_w", (D, 3 * D))
    din("qkv_b", (128, 18))
    din("proj_w", (D, D))
    din("proj_b", (128, KT))
    din("ln1_g", (128, KT))
    din("ln1_b", (128, KT))
    din("ln2_g", (128, KT))
    din("ln2_b", (128, KT))
    din("gate_w", (D, E))
    din("w1", (D, HID))
    din("b1", (128, HM))
    din("w2", (HID, D))
    din("b2", (1, D))
    din("tok0", (1, 1), I32)
    din("eoh", (128, 1))

    outs = {"out": nc.dram_tensor("out", [N, D], F32,
                                  kind="ExternalOutput").ap()}
    if debug:
        outs["dbg"] = {
            nm: nc.dram_tensor(f"dbg_{nm}", list(shp),
                               I16 if nm in ("cidx", "didx") else F32,
                               kind="ExternalOutput").ap()
            for nm, shp in DBG_SPECS.items()
        }

    with tile.TileContext(nc) as tc:
        build_block(tc, outs, ins)
    nc.compile()
    _PROGRAM_CACHE[key] = nc
    return nc


def round_f32r(x):
    """Round fp32 to fp32r (8e11m) with round-to-nearest-even."""
    b = np.ascontiguousarray(x, np.float32).view(np.uint32).copy()
    lsb = (b >> np.uint32(12)) & np.uint32(1)
    r = b + np.uint32(0x7FF) + lsb
    return (r & np.uint32(0xFFFFF000)).view(np.float32)


def prep_in_maps(inputs):
    f = {k: np.ascontiguousarray(np.asarray(v), dtype=np.float32)
         for k, v in inputs.items()}
    share = {
        "qkv_w": f["qkv_w"],
        "qkv_b": f["qkv_b"].reshape(18, 128).T.copy(),
        "proj_w": f["proj_w"],
        "proj_b": f["proj_b"].reshape(KT, 128).T.copy(),
        "ln1_g": f["ln1_g"].reshape(KT, 128).T.copy(),
        "ln1_b": f["ln1_b"].reshape(KT, 128).T.copy(),
        "ln2_g": f["ln2_g"].reshape(KT, 128).T.copy(),
        "ln2_b": f["ln2_b"].reshape(KT, 128).T.copy(),
        "gate_w": f["gate_w"],
    }
    in_maps = []
    for o in range(NCORES):
        m = dict(share)
        m["xT"] = f["x"][o].T.copy()
        m["w1"] = round_f32r(f["w1"][o])
        m["b1"] = f["b1"][o].reshape(HM, 128).T.copy()
        m["w2"] = round_f32r(f["w2"][o])
        m["b2"] = f["b2"][o].reshape(1, D).copy()
        m["tok0"] = np.array([[o * N]], np.int32)
        eoh = np.zeros((8, 1), np.float32)
        eoh[o, 0] = 1.0
        m["eoh"] = np.tile(eoh, (16, 1))
        in_maps.append(m)
    return in_maps


def kernel(**inputs):
    nc = get_program(debug=False)
    in_maps = prep_in_maps(inputs)
    res = bass_utils.run_bass_kernel_spmd(
        nc, in_maps, core_ids=list(range(NCORES)), trace=False)
    out = np.stack([r["out"] for r in res.results], axis=0)
    return out.astype(np.float32)


# revision 13
# speedup vs baseline: 1.1686x; 1.1686x over previous
"""MoE transformer block (attention + top-2 routed 8-expert FFN) on 8 TRN2
NeuronCores.

Sharding: data-parallel attention (1 image of 196 tokens per core) +
expert-parallel MoE (1 expert per core). Token dispatch/combine via
AllGather + dma_gather, with slot ranks computed on-device by a cumsum
scan over the routing mask.

Self-contained: hardcodes all shapes; imports only concourse (already on
PYTHONPATH in the runtime image).
"""
import os
import sys
for _p in ("/opt/trn_rl_repo", "/root/.axon_site/_ro/trn_rl_repo"):
    if _p not in sys.path:
        sys.path.append(_p)

import numpy as np
import concourse.bass as bass
import concourse.bacc as bacc
import concourse.mybir as mybir
import concourse.tile as tile
from concourse import masks
from concourse import bass_utils

F32 = mybir.dt.float32
F32R = mybir.dt.float32r
I16 = mybir.dt.int16
I32 = mybir.dt.int32
U32 = mybir.dt.uint32
AF = mybir.ActivationFunctionType
OP = mybir.AluOpType

NCORES = 8
B, N, D = 8, 196, 768
HEADS, HD = 12, 64
E, HID = 8, 3072
T = B * N                  # 1568
C = 512                    # expert capacity (max observed count is 424)
KT = D // 128              # 6 feature k-tiles
HM = HID // 128            # 24 hidden tiles
CM = C // 128              # 4 capacity tiles
LN_EPS = 1e-5
ROWB = 832                 # AG row: 768 xn + 8 logits + pad to 256B multiple
TW = 13                    # ceil(T/128) wrapped token tiles
TWPAD = TW * 128           # 1664
NCH = [128, N - 128]       # valid partitions per token chunk (128 + 68)


def _repl16(nc, dst, src16):
    """Replicate a [16, f] tile across all 128 partitions of dst."""
    nc.vector.memset(dst[:], 0)
    nc.vector.tensor_copy(dst[0:16, :], src16[:])
    nc.vector.stream_shuffle(dst[0:32, :], dst[0:32, :],
                             mask=[i % 16 for i in range(32)])
    nc.vector.tensor_copy(dst[32:64, :], dst[0:32, :])
    nc.vector.tensor_copy(dst[64:128, :], dst[0:64, :])


def build_block(tc, outs, ins):
    nc = tc.nc
    out = outs["out"]          # [196, 768] per-core output
    dbg = outs.get("dbg", {})  # optional {name: AP} debug outputs

    sb = tc.alloc_tile_pool(name="sb", bufs=1)      # small persistents
    sbw = tc.alloc_tile_pool(name="sbw", bufs=3)    # loop workspace
    dram = tc.alloc_tile_pool(name="dram", bufs=1, space="DRAM")

    ident = sb.tile([128, 128], F32)
    masks.make_identity(nc, ident[:])
    ones_col = sb.tile([128, 1], F32)
    nc.vector.memset(ones_col[:], 1.0)
    ones8 = sb.tile([8, 1], F32)
    nc.vector.memset(ones8[:], 1.0)
    eps_sb = sb.tile([1, 1], F32)
    nc.vector.memset(eps_sb[:], LN_EPS)

    cvec = {}
    for nm, w in (("qkv_b", 18), ("proj_b", KT), ("ln1_g", KT), ("ln1_b", KT),
                  ("ln2_g", KT), ("ln2_b", KT), ("b1", HM)):
        t = sb.tile([128, w], F32, tag=nm)
        nc.sync.dma_start(t[:], ins[nm][:])
        cvec[nm] = t
    b2_row = sb.tile([1, D], F32, tag="b2r")
    nc.sync.dma_start(b2_row[:], ins["b2"][:])
    b2_b = sb.tile([128, D], F32, tag="b2b")
    nc.gpsimd.partition_broadcast(b2_b[:], b2_row[:])

    tok0_sb = sb.tile([1, 1], I32, tag="tok0")
    nc.sync.dma_start(tok0_sb[:], ins["tok0"][:])
    eoh_sb = sb.tile([128, 1], F32, tag="eoh")
    nc.sync.dma_start(eoh_sb[:], ins["eoh"][:])

    # medium-lived persistents
    x1T = sb.tile([128, KT, N], F32, tag="x1T")
    x1tok = sb.tile([128, 2, D], F32, tag="x1tok")

    # ---- feature-major LayerNorm via matmul-ones stats -------------------
    def layer_norm_fm(src, g_t, b_t, dst, psln, wk):
        s1 = psln.tile([1, N], F32, tag="lnp1")
        for k in range(KT):
            nc.tensor.matmul(s1[:], ones_col[:], src[:, k, :],
                             start=(k == 0), stop=(k == KT - 1))
        s2 = psln.tile([1, N], F32, tag="lnp2")
        for k in range(KT):
            sq = wk.tile([128, N], F32, tag="ln_sq")
            nc.scalar.activation(sq[:], src[:, k, :], AF.Square)
            nc.tensor.matmul(s2[:], ones_col[:], sq[:],
                             start=(k == 0), stop=(k == KT - 1))
        mu = wk.tile([1, N], F32, tag="ln_mu")
        nc.vector.tensor_scalar_mul(mu[:], s1[:], 1.0 / D)
        nmu2 = wk.tile([1, N], F32, tag="ln_nmu2")
        nc.vector.scalar_tensor_tensor(nmu2[:], mu[:], -1.0, mu[:],
                                       op0=OP.mult, op1=OP.mult)
        var = wk.tile([1, N], F32, tag="ln_var")
        nc.vector.scalar_tensor_tensor(var[:], s2[:], 1.0 / D, nmu2[:],
                                       op0=OP.mult, op1=OP.add)
        sd = wk.tile([1, N], F32, tag="ln_sd")
        nc.scalar.activation(sd[:], var[:], AF.Sqrt, bias=eps_sb[:])
        rstd = wk.tile([1, N], F32, tag="ln_rstd")
        nc.vector.reciprocal(rstd[:], sd[:])
        mu_b = wk.tile([128, N], F32, tag="ln_mub")
        nc.gpsimd.partition_broadcast(mu_b[:], mu[:])
        rstd_b = wk.tile([128, N], F32, tag="ln_rstdb")
        nc.gpsimd.partition_broadcast(rstd_b[:], rstd[:])
        for k in range(KT):
            tmp = wk.tile([128, N], F32, tag="ln_tmp")
            nc.vector.tensor_tensor(tmp[:], src[:, k, :], mu_b[:],
                                    op=OP.subtract)
            nc.vector.tensor_tensor(tmp[:], tmp[:], rstd_b[:], op=OP.mult)
            nc.any.tensor_scalar(dst[:, k, :], tmp[:], g_t[:, k:k + 1],
                                 b_t[:, k:k + 1], op0=OP.mult, op1=OP.add)

    # =====================================================================
    # Phase A: attention (fp32, feature-major)
    # =====================================================================
    with (
        tc.tile_pool(name="p_attn", bufs=1) as pa,
        tc.tile_pool(name="p_qkvw", bufs=1) as pqw,
    ):
        xT_sb = pa.tile([128, KT, N], F32, tag="xT")
        projw_sb = pa.tile([128, KT, D], F32, tag="projw")
        qkvw_sb = pqw.tile([128, KT, 2304], F32, tag="qkvw")
        for k in range(KT):
            sl = slice(128 * k, 128 * (k + 1))
            nc.sync.dma_start(xT_sb[:, k, :], ins["xT"][sl, :])
            nc.sync.dma_start(projw_sb[:, k, :], ins["proj_w"][sl, :])
            nc.sync.dma_start(qkvw_sb[:, k, :], ins["qkv_w"][sl, :])

        xn1 = pa.tile([128, KT, N], F32, tag="xn1")
        with (
            tc.tile_pool(name="psln", bufs=1, space="PSUM") as psln,
            tc.tile_pool(name="wkln1", bufs=2) as wkln1,
        ):
            layer_norm_fm(xT_sb, cvec["ln1_g"], cvec["ln1_b"], xn1, psln,
                          wkln1)

        qkvT = pa.tile([128, 18, N], F32, tag="qkvT")
        with tc.tile_pool(name="psqkv", bufs=3, space="PSUM") as psqkv:
            for m in range(18):
                ps = psqkv.tile([128, N], F32, tag="qkvps")
                for k in range(KT):
                    nc.tensor.matmul(
                        ps[:], qkvw_sb[:, k, 128 * m:128 * (m + 1)],
                        xn1[:, k, :], start=(k == 0), stop=(k == KT - 1))
                nc.any.tensor_scalar(qkvT[:, m, :], ps[:],
                                     cvec["qkv_b"][:, m:m + 1], None,
                                     op0=OP.add)

        def head_slice(base, h):
            return qkvT[64 * (h % 2):64 * (h % 2) + 64, base + h // 2, :]

        attn_out = pa.tile([128, 2, D], F32, tag="attn_out")  # token-major
        with (
            tc.tile_pool(name="pss", bufs=2, space="PSUM") as pss,
            tc.tile_pool(name="pst", bufs=3, space="PSUM") as pst,
            tc.tile_pool(name="pso", bufs=2, space="PSUM") as pso,
            tc.tile_pool(name="wka", bufs=3) as wka,
        ):
            for h in range(HEADS):
                qT, kT, vT = (head_slice(b, h) for b in (0, 6, 12))
                b0 = HD * (h % 2)
                v_tok = wka.tile([128, 2, HD], F32, tag="v_tok")
                for i, sz in enumerate(NCH):
                    pv = pst.tile([128, 128], F32, tag="ptt")
                    nc.tensor.transpose(pv[:sz, :HD],
                                        vT[:, 128 * i:128 * i + sz],
                                        ident[b0:b0 + HD, b0:b0 + HD])
                    nc.any.tensor_copy(v_tok[:sz, i, :], pv[:sz, :HD])
                attnT = wka.tile([128, 2, N], F32, tag="attnT")
                rcp = wka.tile([128, 2], F32, tag="rcp")
                for i, sz in enumerate(NCH):
                    ps = pss.tile([128, N], F32, tag="sps")
                    nc.tensor.matmul(ps[:sz, :], qT[:, 128 * i:128 * i + sz],
                                     kT[:], start=True, stop=True)
                    m0 = wka.tile([128, 1], F32, tag="sm_m0")
                    nc.vector.tensor_reduce(m0[:sz], ps[:sz, :],
                                            axis=mybir.AxisListType.X,
                                            op=OP.max)
                    m0n = wka.tile([128, 1], F32, tag="sm_m0n")
                    nc.vector.tensor_scalar_mul(m0n[:sz], m0[:sz], -0.125)
                    ex = wka.tile([128, N], F32, tag="sm_ex")
                    ssum = wka.tile([128, 1], F32, tag="sm_sum")
                    nc.scalar.activation(ex[:sz, :], ps[:sz, :], AF.Exp,
                                         bias=m0n[:sz], scale=0.125,
                                         accum_out=ssum[:sz])
                    nc.vector.reciprocal(rcp[:sz, i:i + 1], ssum[:sz])
                    for j, szj in enumerate(NCH):
                        pt = pst.tile([128, 128], F32, tag="ptt")
                        nc.tensor.transpose(pt[:szj, :sz],
                                            ex[:sz, 128 * j:128 * j + szj],
                                            ident[0:sz, 0:sz])
                        nc.any.tensor_copy(
                            attnT[:szj, j, 128 * i:128 * i + sz],
                            pt[:szj, :sz])
                for i, sz in enumerate(NCH):
                    po = pso.tile([128, HD], F32, tag="pso")
                    for j, szj in enumerate(NCH):
                        nc.tensor.matmul(po[:sz, :],
                                         attnT[:szj, j, 128 * i:128 * i + sz],
                                         v_tok[:szj, j, :],
                                         start=(j == 0), stop=(j == 1))
                    nc.any.tensor_scalar(
                        attn_out[:sz, i, 64 * h:64 * (h + 1)], po[:sz, :],
                        rcp[:sz, i:i + 1], None, op0=OP.mult)

        aoT = pa.tile([128, KT, N], F32, tag="aoT")
        with (
            tc.tile_pool(name="psat", bufs=3, space="PSUM") as psat,
            tc.tile_pool(name="psp", bufs=3, space="PSUM") as psp,
        ):
            for ft in range(KT):
                pt = psat.tile([128, N], F32, tag="psatt")
                for i, sz in enumerate(NCH):
                    nc.tensor.transpose(
                        pt[:, 128 * i:128 * i + sz],
                        attn_out[:sz, i, 128 * ft:128 * (ft + 1)],
                        ident[0:sz, 0:sz])
                nc.any.tensor_copy(aoT[:, ft, :], pt[:])
            for ft in range(KT):
                ps = psp.tile([128, N], F32, tag="pspp")
                for k in range(KT):
                    nc.tensor.matmul(ps[:],
                                     projw_sb[:, k, 128 * ft:128 * (ft + 1)],
                                     aoT[:, k, :], start=(k == 0),
                                     stop=(k == KT - 1))
                nc.vector.scalar_tensor_tensor(
                    x1T[:, ft, :], ps[:], cvec["proj_b"][:, ft:ft + 1],
                    xT_sb[:, ft, :], op0=OP.add, op1=OP.add)

    # =====================================================================
    # Phase B: LN2, bounce assembly, AllGather #1
    # =====================================================================
    ag_in = dram.tile([N, ROWB], F32)
    ag_out = dram.tile([T, ROWB], F32, addr_space="Shared")
    with tc.tile_pool(name="p_gate", bufs=1) as pg:
        gatew_sb = pg.tile([128, KT, 8], F32, tag="gatew")
        for k in range(KT):
            nc.sync.dma_start(gatew_sb[:, k, :],
                              ins["gate_w"][128 * k:128 * (k + 1), :])
        xnT = pg.tile([128, KT, N], F32, tag="xnT")
        with (
            tc.tile_pool(name="psln2", bufs=1, space="PSUM") as psln2,
            tc.tile_pool(name="wkln2", bufs=2) as wkln2,
        ):
            layer_norm_fm(x1T, cvec["ln2_g"], cvec["ln2_b"], xnT, psln2,
                          wkln2)

        bounce = pg.tile([128, 2, ROWB], F32, tag="bounce")
        nc.vector.memset(bounce[:, :, D + 8:ROWB], 0.0)
        with tc.tile_pool(name="psb", bufs=2, space="PSUM") as psb:
            for i, sz in enumerate(NCH):
                pt = psb.tile([128, D], F32, tag="psbt")
                for ft in range(KT):
                    nc.tensor.transpose(pt[:sz, 128 * ft:128 * (ft + 1)],
                                        xnT[:, ft, 128 * i:128 * i + sz],
                                        ident[:])
                nc.any.tensor_copy(bounce[:sz, i, 0:D], pt[:sz, :])
                pl = psb.tile([128, 8], F32, tag="psbl")
                for k in range(KT):
                    nc.tensor.matmul(pl[:sz, :],
                                     xnT[:, k, 128 * i:128 * i + sz],
                                     gatew_sb[:, k, :], start=(k == 0),
                                     stop=(k == KT - 1))
                nc.any.tensor_copy(bounce[:sz, i, D:D + 8], pl[:sz, :])
        for i, sz in enumerate(NCH):
            nc.sync.dma_start(ag_in[128 * i:128 * i + sz, :],
                              bounce[:sz, i, :])
        if "xnT" in dbg:
            for k in range(KT):
                nc.sync.dma_start(dbg["xnT"][128 * k:128 * (k + 1), :],
                                  xnT[:, k, :])
    nc.gpsimd.collective_compute(
        "AllGather", OP.bypass, ins=[ag_in.opt()], outs=[ag_out.opt()],
        replica_groups=[list(range(NCORES))])

    # =====================================================================
    # Phase C: routing (replicated on every core)
    # =====================================================================
    lg = sb.tile([128, TW, 8], F32, tag="lg")
    nc.vector.memset(lg[:], 0.0)
    nc.sync.dma_start(
        lg[:, 0:12, :],
        ag_out[0:1536, D:D + 8].rearrange("(i p) c -> p i c", p=128))
    nc.sync.dma_start(lg[0:32, 12, :], ag_out[1536:T, D:D + 8])

    gates = sb.tile([128, TW, 8], F32, tag="gates")
    g0wr = sb.tile([128, TW], F32, tag="g0wr")
    for i in range(TW):
        lgi = lg[:, i, :]
        m0 = sbw.tile([128, 1], F32, tag="rt_m0")
        nc.vector.tensor_reduce(m0[:], lgi, axis=mybir.AxisListType.X,
                                op=OP.max)
        m0n = sbw.tile([128, 1], F32, tag="rt_m0n")
        nc.vector.tensor_scalar_mul(m0n[:], m0[:], -1.0)
        mk0 = sbw.tile([128, 8], F32, tag="rt_mk0")
        nc.vector.tensor_scalar(mk0[:], lgi, m0[:], None, op0=OP.is_ge)
        msk = sbw.tile([128, 8], F32, tag="rt_msk")
        nc.vector.scalar_tensor_tensor(msk[:], mk0[:], -1e30, lgi,
                                       op0=OP.mult, op1=OP.add)
        m1 = sbw.tile([128, 1], F32, tag="rt_m1")
        nc.vector.tensor_reduce(m1[:], msk[:], axis=mybir.AxisListType.X,
                                op=OP.max)
        ex = sbw.tile([128, 8], F32, tag="rt_ex")
        nc.scalar.activation(ex[:], lgi, AF.Exp, bias=m0n[:])
        e1 = sbw.tile([128, 1], F32, tag="rt_e1")
        nc.scalar.activation(e1[:], m1[:], AF.Exp, bias=m0n[:])
        den = sbw.tile([128, 1], F32, tag="rt_den")
        nc.vector.tensor_scalar_add(den[:], e1[:], 1.0)
        nc.vector.reciprocal(g0wr[:, i:i + 1], den[:])
        mt = sbw.tile([128, 8], F32, tag="rt_mt")
        nc.vector.tensor_scalar(mt[:], lgi, m1[:], None, op0=OP.is_ge)
        gu = sbw.tile([128, 8], F32, tag="rt_gu")
        nc.vector.tensor_tensor(gu[:], ex[:], mt[:], op=OP.mult)
        nc.vector.tensor_scalar(gates[:, i, :], gu[:], g0wr[:, i:i + 1],
                                None, op0=OP.mult)

    p_rt = tc.alloc_tile_pool(name="p_rt", bufs=1)
    g8_t = p_rt.tile([8, TWPAD], F32, tag="g8")
    mask8_t = p_rt.tile([8, TWPAD], F32, tag="mask8")
    rank8_t = p_rt.tile([8, TWPAD], F32, tag="rank8")
    slotA_t = p_rt.tile([8, TWPAD], F32, tag="slotA")
    sel0_t = p_rt.tile([8, TWPAD], F32, tag="sel0")
    sel1_t = p_rt.tile([8, TWPAD], F32, tag="sel1")
    s0A_t = p_rt.tile([8, TWPAD], F32, tag="s0A")
    s1A_t = p_rt.tile([8, TWPAD], F32, tag="s1A")
    s0G_t = p_rt.tile([8, TWPAD], F32, tag="s0G")
    mrow_t = p_rt.tile([1, TWPAD], F32, tag="mrow")
    g8 = g8_t[:]
    mask8 = mask8_t[:]
    rank8 = rank8_t[:]
    slotA = slotA_t[:]
    sel0 = sel0_t[:]
    sel1 = sel1_t[:]
    s0A = s0A_t[:]
    s1A = s1A_t[:]
    s0G = s0G_t[:]
    mrow = mrow_t[:]
    with tc.tile_pool(name="psg", bufs=3, space="PSUM") as psg:
        for i in range(TW):
            pt = psg.tile([8, 128], F32, tag="psgt")
            nc.tensor.transpose(pt[:], gates[:, i, :], ident[:])
            nc.any.tensor_copy(g8[:, 128 * i:128 * (i + 1)], pt[:])

    nc.vector.tensor_scalar(mask8, g8, 0.0, None, op0=OP.is_gt)
    nc.vector.tensor_tensor_scan(rank8, mask8, mask8, initial=0.0,
                                 op0=OP.add, op1=OP.bypass)
    eoff_i = sb.tile([8, 1], I32, tag="eoff_i")
    nc.gpsimd.iota(eoff_i[:], pattern=[[1, 1]], base=-1, channel_multiplier=C)
    eoff = sb.tile([8, 1], F32, tag="eoff")
    nc.vector.tensor_copy(eoff[:], eoff_i[:])
    nc.vector.tensor_scalar(slotA, rank8, eoff[:], None, op0=OP.add)
    nc.vector.tensor_scalar(sel0, g8, 0.5, None, op0=OP.is_ge)
    nc.vector.tensor_tensor(sel1, mask8, sel0, op=OP.subtract)
    nc.vector.tensor_tensor(s0A, sel0, slotA, op=OP.mult)
    nc.vector.tensor_tensor(s1A, sel1, slotA, op=OP.mult)
    nc.vector.tensor_tensor(s0G, sel0, g8, op=OP.mult)

    # my-token window [tok0, tok0+196) via dynamic rhs slice
    treg = nc.tensor.alloc_register("tok0reg")
    nc.tensor.reg_load(treg, tok0_sb[0:1, 0:1])
    toff = nc.tensor.snap(treg, donate=True, min_val=0,
                          max_val=N * (NCORES - 1))

    c0my16 = sb.tile([1, N], I16, tag="c0my16")
    c1my16 = sb.tile([1, N], I16, tag="c1my16")
    g0my = sb.tile([1, N], F32, tag="g0my")
    with tc.tile_pool(name="psc", bufs=3, space="PSUM") as psc:
        for srcv, base, dst in ((s0A, 0, c0my16), (s1A, 0, c1my16),
                                (s0G, 0, g0my)):
            p = psc.tile([1, N], F32, tag="pscc")
            nc.tensor.matmul(p[:], ones_col[base:base + 8, 0:1],
                             srcv[:, bass.ds(toff, N)],
                             start=True, stop=True)
            nc.any.tensor_copy(dst[:], p[:])

    # combine idx: [c0 | pad | c1 | pad] -> dram[512] -> wrapped16 -> 128p
    c01 = sb.tile([1, 512], I16, tag="c01")
    nc.vector.memset(c01[:], 0)
    nc.vector.tensor_copy(c01[:, 0:N], c0my16[:])
    nc.vector.tensor_copy(c01[:, 256:256 + N], c1my16[:])
    cidx_d = dram.tile([1, 512], I16)
    nc.sync.dma_start(cidx_d[:], c01[:])
    cidx16 = sb.tile([16, 32], I16, tag="cidx16")
    nc.sync.dma_start(cidx16[:], cidx_d[:].rearrange("1 (f p) -> p f", p=16))
    cidx128 = sb.tile([128, 32], I16, tag="cidx128")
    _repl16(nc, cidx128, cidx16)

    # g0 window -> wrapped [128, 2]
    g0_d = dram.tile([1, N], F32)
    nc.sync.dma_start(g0_d[:], g0my[:])
    g0w2 = sb.tile([128, 2], F32, tag="g0w2")
    nc.vector.memset(g0w2[:], 0.0)
    nc.sync.dma_start(g0w2[:, 0:1], g0_d[0:1, 0:128].rearrange("1 p -> p 1"))
    nc.sync.dma_start(g0w2[0:N - 128, 1:2],
                      g0_d[0:1, 128:N].rearrange("1 p -> p 1"))

    # dispatch idx: my expert row of mask8 -> compacted token list
    with tc.tile_pool(name="psm", bufs=2, space="PSUM") as psm:
        for c0, cw in ((0, 512), (512, 512), (1024, 512), (1536, 128)):
            p = psm.tile([1, cw], F32, tag="psmm")
            nc.tensor.matmul(p[:], eoh_sb[0:8, 0:1], mask8[:, c0:c0 + cw],
                             start=True, stop=True)
            nc.any.tensor_copy(mrow[0:1, c0:c0 + cw], p[:])
    mrow_d = dram.tile([1, TWPAD], F32)
    nc.sync.dma_start(mrow_d[:], mrow)
    m16 = sb.tile([16, T // 16], F32, tag="m16")
    nc.sync.dma_start(m16[:],
                      mrow_d[0:1, 0:T].rearrange("1 (f p) -> p f", p=16))
    iot = sb.tile([16, T // 16], I32, tag="iot")
    nc.gpsimd.iota(iot[:], pattern=[[16, T // 16]], base=0,
                   channel_multiplier=1)
    iotf = sb.tile([16, T // 16], F32, tag="iotf")
    nc.vector.tensor_copy(iotf[:], iot[:])
    cand = sb.tile([16, T // 16], F32, tag="cand")
    nc.vector.tensor_scalar_add(cand[:], iotf[:], 1.0)
    nc.vector.tensor_tensor(cand[:], cand[:], m16[:], op=OP.mult)
    nc.vector.tensor_scalar_add(cand[:], cand[:], -1.0)
    nfound = sb.tile([1, 1], U32, tag="nfound")
    comp = sb.tile([16, C // 16], F32, tag="comp")
    nc.gpsimd.sparse_gather(comp[:], cand[:], num_found=nfound[:])
    nc.vector.tensor_scalar(comp[:], comp[:], float(T - 1), 0.0,
                            op0=OP.min, op1=OP.max)
    didx16 = sb.tile([16, C // 16], I16, tag="didx16")
    nc.vector.tensor_copy(didx16[:], comp[:])
    didx128 = sb.tile([128, C // 16], I16, tag="didx128")
    _repl16(nc, didx128, didx16)

    if "rank8" in dbg:
        nc.sync.dma_start(dbg["rank8"][:], rank8)
    p_rt.release()

    # =====================================================================
    # Phase D: dispatch gather, expert FFN, AllGather #2
    # =====================================================================
    y_d = dram.tile([C, D], F32)
    p_hT = tc.alloc_tile_pool(name="p_hT", bufs=1)
    with tc.tile_pool(name="p_w1", bufs=1) as pw1:
        w1_sb = pw1.tile([128, KT, HID], F32R, tag="w1")
        for k in range(KT):
            nc.sync.dma_start(w1_sb[:, k, :],
                              ins["w1"][128 * k:128 * (k + 1), :]
                              .bitcast(F32R))
        XgT = pw1.tile([128, KT, C], F32R, tag="XgT")
        with tc.tile_pool(name="p_xg", bufs=1) as pxg:
            Xg = pxg.tile([128, CM, D], F32, tag="Xg")
            nc.gpsimd.dma_gather(Xg[:], ag_out[:, 0:D], didx128[:],
                                 num_idxs=C, num_idxs_reg=C, elem_size=D,
                                 elem_step=ROWB)
            with tc.tile_pool(name="psx", bufs=3, space="PSUM") as psx:
                for ct in range(CM):
                    for ft in range(KT):
                        pt = psx.tile([128, 128], F32, tag="psxt")
                        nc.tensor.transpose(
                            pt[:], Xg[:, ct, 128 * ft:128 * (ft + 1)],
                            ident[:])
                        nc.any.tensor_copy(
                            XgT[:, ft, 128 * ct:128 * (ct + 1)], pt[:])

        hT = p_hT.tile([128, HM, C], F32R, tag="hT")
        with tc.tile_pool(name="ps1", bufs=3, space="PSUM") as ps1:
            for m in range(HM):
                ph = ps1.tile([128, C], F32, tag="ps1t")
                for k in range(KT):
                    nc.tensor.matmul(ph[:],
                                     w1_sb[:, k, 128 * m:128 * (m + 1)],
                                     XgT[:, k, :], start=(k == 0),
                                     stop=(k == KT - 1))
                nc.scalar.activation(hT[:, m, :], ph[:], AF.Gelu,
                                     bias=cvec["b1"][:, m:m + 1])

    with (
        tc.tile_pool(name="p_y", bufs=1) as py,
        tc.tile_pool(name="ps4", bufs=1, space="PSUM") as ps4,
        tc.tile_pool(name="w2p", bufs=3) as w2p,
    ):
        y_sb = py.tile([128, CM, D], F32, tag="y_sb")
        yps = {}
        for mc in range(CM):
            for nb in range(2):
                ypst = ps4.tile([128, 384], F32, tag=f"y{mc}{nb}")
                yps[(mc, nb)] = ypst
        for k in range(HM):
            w2t = w2p.tile([128, D], F32R, tag="w2t")
            nc.sync.dma_start(w2t[:],
                              ins["w2"][128 * k:128 * (k + 1), :]
                              .bitcast(F32R))
            for mc in range(CM):
                for nb in range(2):
                    nc.tensor.matmul(
                        yps[(mc, nb)][:],
                        hT[:, k, 128 * mc:128 * (mc + 1)],
                        w2t[:, 384 * nb:384 * (nb + 1)],
                        start=(k == 0), stop=(k == HM - 1))
        for mc in range(CM):
            for nb in range(2):
                nc.any.tensor_copy(y_sb[:, mc, 384 * nb:384 * (nb + 1)],
                                   yps[(mc, nb)][:])
        nc.sync.dma_start(y_d[:].rearrange("(i p) d -> p i d", p=128),
                          y_sb[:])
        if "ysb" in dbg:
            nc.sync.dma_start(
                dbg["ysb"][:].rearrange("(i p) d -> p i d", p=128), y_sb[:])

    p_hT.release()
    yall = dram.tile([NCORES * C, D], F32, addr_space="Shared")
    nc.gpsimd.collective_compute(
        "AllGather", OP.bypass, ins=[y_d.opt()], outs=[yall.opt()],
        replica_groups=[list(range(NCORES))])

    # x1 token-major + b2 (independent of expert compute; overlaps)
    with tc.tile_pool(name="psr", bufs=2, space="PSUM") as psr:
        for i, sz in enumerate(NCH):
            pt = psr.tile([128, D], F32, tag="psrt")
            for ft in range(KT):
                nc.tensor.transpose(pt[:sz, 128 * ft:128 * (ft + 1)],
                                    x1T[:, ft, 128 * i:128 * i + sz],
                                    ident[:])
            nc.vector.tensor_tensor(x1tok[:sz, i, :], pt[:sz, :],
                                    b2_b[:sz, :], op=OP.add)

    # =====================================================================
    # Phase E: combine gather + weighted sum + output
    # =====================================================================
    with tc.tile_pool(name="p_e", bufs=1) as pe:
        g_tok = pe.tile([128, 4, D], F32, tag="g_tok")
        nc.gpsimd.dma_gather(g_tok[:], yall[:], cidx128[:],
                             num_idxs=512, num_idxs_reg=512, elem_size=D)
        for i, sz in enumerate(NCH):
            dtile = sbw.tile([128, D], F32, tag="cmb_d")
            nc.vector.tensor_tensor(dtile[:sz, :], g_tok[:sz, i, :],
                                    g_tok[:sz, i + 2, :], op=OP.subtract)
            t2 = sbw.tile([128, D], F32, tag="cmb_t2")
            nc.vector.scalar_tensor_tensor(t2[:sz, :], dtile[:sz, :],
                                           g0w2[:sz, i:i + 1],
                                           g_tok[:sz, i + 2, :],
                                           op0=OP.mult, op1=OP.add)
            ocmb = sbw.tile([128, D], F32, tag="cmb_o")
            nc.vector.tensor_tensor(ocmb[:sz, :], t2[:sz, :],
                                    x1tok[:sz, i, :], op=OP.add)
            nc.sync.dma_start(out[128 * i:128 * i + sz, :], ocmb[:sz, :])

    # remaining debug taps
    if "gates" in dbg:
        for i in range(TW):
            nc.sync.dma_start(dbg["gates"][0:128, 8 * i:8 * (i + 1)],
                              gates[:, i, :])
    if "cidx" in dbg:
        nc.sync.dma_start(dbg["cidx"][:], cidx128[0:16, :])
    if "didx" in dbg:
        nc.sync.dma_start(dbg["didx"][:], didx128[0:16, :])
    if "g0w2" in dbg:
        nc.sync.dma_start(dbg["g0w2"][:], g0w2[:])
    if "x1T" in dbg:
        for k in range(KT):
            nc.sync.dma_start(dbg["x1T"][128 * k:128 * (k + 1), :],
                              x1T[:, k, :])

    sbw.release()
    sb.release()
    dram.release()


# ---------------------------------------------------------------------------
# host side
# ---------------------------------------------------------------------------

DBG_SPECS = {
    "xnT": (D, N), "gates": (128, TW * 8), "rank8": (8, TWPAD),
    "cidx": (16, 32), "didx": (16, C // 16), "g0w2": (128, 2),
    "ysb": (C, D), "x1T": (D, N),
}

_PROGRAM_CACHE = {}


def get_program(debug=False):
    key = bool(debug)
    if key in _PROGRAM_CACHE:
        return _PROGRAM_CACHE[key]
    nc = bacc.Bacc("TRN2", target_bir_lowering=False, debug=False,
                   enable_asserts=True, num_devices=NCORES)
    ins = {}

    def din(name, shape, dtype=F32):
        ins[name] = nc.dram_tensor(name, list(shape), dtype,
                                   kind="ExternalInput").ap()

    din("xT", (D, N))
    din("qkv_w", (D, 3 * D))
    din("qkv_b", (128, 18))
    din("proj_w", (D, D))
    din("proj_b", (128, KT))
    din("ln1_g", (128, KT))
    din("ln1_b", (128, KT))
    din("ln2_g", (128, KT))
    din("ln2_b", (128, KT))
    din("gate_w", (D, E))
    din("w1", (D, HID))
    din("b1", (128, HM))
    din("w2", (HID, D))
    din("b2", (1, D))
    din("tok0", (1, 1), I32)
    din("eoh", (128, 1))

    outs = {"out": nc.dram_tensor("out", [N, D], F32,
                                  kind="ExternalOutput").ap()}
    if debug:
        outs["dbg"] = {
            nm: nc.dram_tensor(f"dbg_{nm}", list(shp),
                               I16 if nm in ("cidx", "didx") else F32,
                               kind="ExternalOutput").ap()
            for nm, shp in DBG_SPECS.items()
        }

    with tile.TileContext(nc) as tc:
        build_block(tc, outs, ins)
    nc.compile()
    _PROGRAM_CACHE[key] = nc
    return nc


def round_f32r(x):
    """Round fp32 to fp32r (8e11m) with round-to-nearest-even."""
    b = np.ascontiguousarray(x, np.float32).view(np.uint32).copy()
    lsb = (b >> np.uint32(12)) & np.uint32(1)
    r = b + np.uint32(0x7FF) + lsb
    return (r & np.uint32(0xFFFFF000)).view(np.float32)


def prep_in_maps(inputs):
    f = {k: np.ascontiguousarray(np.asarray(v), dtype=np.float32)
         for k, v in inputs.items()}
    share = {
        "qkv_w": f["qkv_w"],
        "qkv_b": f["qkv_b"].reshape(18, 128).T.copy(),
        "proj_w": f["proj_w"],
        "proj_b": f["proj_b"].reshape(KT, 128).T.copy(),
        "ln1_g": f["ln1_g"].reshape(KT, 128).T.copy(),
        "ln1_b": f["ln1_b"].reshape(KT, 128).T.copy(),
        "ln2_g": f["ln2_g"].reshape(KT, 128).T.copy(),
        "ln2_b": f["ln2_b"].reshape(KT, 128).T.copy(),
        "gate_w": f["gate_w"],
    }
    in_maps = []
    for o in range(NCORES):
        m = dict(share)
        m["xT"] = f["x"][o].T.copy()
        m["w1"] = round_f32r(f["w1"][o])
        m["b1"] = f["b1"][o].reshape(HM, 128).T.copy()
        m["w2"] = round_f32r(f["w2"][o])
        m["b2"] = f["b2"][o].reshape(1, D).copy()
        m["tok0"] = np.array([[o * N]], np.int32)
        eoh = np.zeros((8, 1), np.float32)
        eoh[o, 0] = 1.0
        m["eoh"] = np.tile(eoh, (16, 1))
        in_maps.append(m)
    return in_maps


def kernel(**inputs):
    nc = get_program(debug=False)
    in_maps = prep_in_maps(inputs)
    res = bass_utils.run_bass_kernel_spmd(
        nc, in_maps, core_ids=list(range(NCORES)), trace=False)
    out = np.stack([r["out"] for r in res.results], axis=0)
    return out.astype(np.float32)


# revision 15
# speedup vs baseline: 1.2392x; 1.0604x over previous
"""MoE transformer block (attention + top-2 routed 8-expert FFN) on 8 TRN2
NeuronCores.

Sharding: data-parallel attention (1 image of 196 tokens per core) +
expert-parallel MoE (1 expert per core). Token dispatch/combine via
AllGather + dma_gather, with slot ranks computed on-device by a cumsum
scan over the routing mask.

Self-contained: hardcodes all shapes; imports only concourse (already on
PYTHONPATH in the runtime image).
"""
import os
import sys
for _p in ("/opt/trn_rl_repo", "/root/.axon_site/_ro/trn_rl_repo"):
    if _p not in sys.path:
        sys.path.append(_p)

import numpy as np
import concourse.bass as bass
import concourse.bacc as bacc
import concourse.mybir as mybir
import concourse.tile as tile
from concourse import masks
from concourse import bass_utils

F32 = mybir.dt.float32
F32R = mybir.dt.float32r
I16 = mybir.dt.int16
I32 = mybir.dt.int32
U32 = mybir.dt.uint32
AF = mybir.ActivationFunctionType
OP = mybir.AluOpType

NCORES = 8
B, N, D = 8, 196, 768
HEADS, HD = 12, 64
E, HID = 8, 3072
T = B * N                  # 1568
C = 512                    # expert capacity (max observed count is 424)
KT = D // 128              # 6 feature k-tiles
HM = HID // 128            # 24 hidden tiles
CM = C // 128              # 4 capacity tiles
LN_EPS = 1e-5
ROWB = 832                 # AG row: 768 xn + 8 logits + pad to 256B multiple
TW = 13                    # ceil(T/128) wrapped token tiles
TWPAD = TW * 128           # 1664
NCH = [128, N - 128]       # valid partitions per token chunk (128 + 68)


def _repl16(nc, dst, src16):
    """Replicate a [16, f] tile across all 128 partitions of dst."""
    nc.vector.memset(dst[:], 0)
    nc.vector.tensor_copy(dst[0:16, :], src16[:])
    nc.vector.stream_shuffle(dst[0:32, :], dst[0:32, :],
                             mask=[i % 16 for i in range(32)])
    nc.vector.tensor_copy(dst[32:64, :], dst[0:32, :])
    nc.vector.tensor_copy(dst[64:128, :], dst[0:64, :])


def build_block(tc, outs, ins):
    nc = tc.nc
    out = outs["out"]          # [196, 768] per-core output
    dbg = outs.get("dbg", {})  # optional {name: AP} debug outputs

    sb = tc.alloc_tile_pool(name="sb", bufs=1)      # small persistents
    sbw = tc.alloc_tile_pool(name="sbw", bufs=3)    # loop workspace
    dram = tc.alloc_tile_pool(name="dram", bufs=1, space="DRAM")

    ident = sb.tile([128, 128], F32)
    masks.make_identity(nc, ident[:])
    ones_col = sb.tile([128, 1], F32)
    nc.vector.memset(ones_col[:], 1.0)
    ones8 = sb.tile([8, 1], F32)
    nc.vector.memset(ones8[:], 1.0)
    eps_sb = sb.tile([1, 1], F32)
    nc.vector.memset(eps_sb[:], LN_EPS)

    cvec = {}
    for nm, w in (("qkv_b", 18), ("proj_b", KT), ("ln1_g", KT), ("ln1_b", KT),
                  ("ln2_g", KT), ("ln2_b", KT), ("b1", HM)):
        t = sb.tile([128, w], F32, tag=nm)
        nc.sync.dma_start(t[:], ins[nm][:])
        cvec[nm] = t
    b2_row = sb.tile([1, D], F32, tag="b2r")
    nc.sync.dma_start(b2_row[:], ins["b2"][:])
    b2_b = sb.tile([128, D], F32, tag="b2b")
    nc.gpsimd.partition_broadcast(b2_b[:], b2_row[:])

    tok0_sb = sb.tile([1, 1], I32, tag="tok0")
    nc.sync.dma_start(tok0_sb[:], ins["tok0"][:])
    eoh_sb = sb.tile([128, 1], F32, tag="eoh")
    nc.sync.dma_start(eoh_sb[:], ins["eoh"][:])

    # medium-lived persistents
    x1T = sb.tile([128, KT, N], F32, tag="x1T")
    x1tok = sb.tile([128, 2, D], F32, tag="x1tok")

    # ---- feature-major LayerNorm via matmul-ones stats -------------------
    def layer_norm_fm(src, g_t, b_t, dst, psln, wk):
        s1 = psln.tile([1, N], F32, tag="lnp1")
        for k in range(KT):
            nc.tensor.matmul(s1[:], ones_col[:], src[:, k, :],
                             start=(k == 0), stop=(k == KT - 1))
        s2 = psln.tile([1, N], F32, tag="lnp2")
        for k in range(KT):
            sq = wk.tile([128, N], F32, tag="ln_sq")
            nc.scalar.activation(sq[:], src[:, k, :], AF.Square)
            nc.tensor.matmul(s2[:], ones_col[:], sq[:],
                             start=(k == 0), stop=(k == KT - 1))
        mu = wk.tile([1, N], F32, tag="ln_mu")
        nc.vector.tensor_scalar_mul(mu[:], s1[:], 1.0 / D)
        nmu2 = wk.tile([1, N], F32, tag="ln_nmu2")
        nc.vector.scalar_tensor_tensor(nmu2[:], mu[:], -1.0, mu[:],
                                       op0=OP.mult, op1=OP.mult)
        var = wk.tile([1, N], F32, tag="ln_var")
        nc.vector.scalar_tensor_tensor(var[:], s2[:], 1.0 / D, nmu2[:],
                                       op0=OP.mult, op1=OP.add)
        sd = wk.tile([1, N], F32, tag="ln_sd")
        nc.scalar.activation(sd[:], var[:], AF.Sqrt, bias=eps_sb[:])
        rstd = wk.tile([1, N], F32, tag="ln_rstd")
        nc.vector.reciprocal(rstd[:], sd[:])
        mu_b = wk.tile([128, N], F32, tag="ln_mub")
        nc.gpsimd.partition_broadcast(mu_b[:], mu[:])
        rstd_b = wk.tile([128, N], F32, tag="ln_rstdb")
        nc.gpsimd.partition_broadcast(rstd_b[:], rstd[:])
        for k in range(KT):
            tmp = wk.tile([128, N], F32, tag="ln_tmp")
            nc.vector.tensor_tensor(tmp[:], src[:, k, :], mu_b[:],
                                    op=OP.subtract)
            nc.vector.tensor_tensor(tmp[:], tmp[:], rstd_b[:], op=OP.mult)
            nc.any.tensor_scalar(dst[:, k, :], tmp[:], g_t[:, k:k + 1],
                                 b_t[:, k:k + 1], op0=OP.mult, op1=OP.add)

    # =====================================================================
    # Phase A: attention (fp32, feature-major)
    # =====================================================================
    with (
        tc.tile_pool(name="p_attn", bufs=1) as pa,
        tc.tile_pool(name="p_qkvw", bufs=1) as pqw,
    ):
        xT_sb = pa.tile([128, KT, N], F32, tag="xT")
        projw_sb = pa.tile([128, KT, D], F32, tag="projw")
        qkvw_sb = pqw.tile([128, KT, 2304], F32, tag="qkvw")
        for k in range(KT):
            nc.sync.dma_start(xT_sb[:, k, :],
                              ins["xT"][128 * k:128 * (k + 1), :])
        for k in range(KT):
            nc.sync.dma_start(qkvw_sb[:, k, :],
                              ins["qkv_w"][128 * k:128 * (k + 1), :])
        for k in range(KT):
            nc.sync.dma_start(projw_sb[:, k, :],
                              ins["proj_w"][128 * k:128 * (k + 1), :])

        xn1 = pa.tile([128, KT, N], F32, tag="xn1")
        with (
            tc.tile_pool(name="psln", bufs=1, space="PSUM") as psln,
            tc.tile_pool(name="wkln1", bufs=2) as wkln1,
        ):
            layer_norm_fm(xT_sb, cvec["ln1_g"], cvec["ln1_b"], xn1, psln,
                          wkln1)

        qkvT = pa.tile([128, 18, N], F32, tag="qkvT")
        with tc.tile_pool(name="psqkv", bufs=3, space="PSUM") as psqkv:
            for m in range(18):
                ps = psqkv.tile([128, N], F32, tag="qkvps")
                for k in range(KT):
                    nc.tensor.matmul(
                        ps[:], qkvw_sb[:, k, 128 * m:128 * (m + 1)],
                        xn1[:, k, :], start=(k == 0), stop=(k == KT - 1))
                nc.any.tensor_scalar(qkvT[:, m, :], ps[:],
                                     cvec["qkv_b"][:, m:m + 1], None,
                                     op0=OP.add)

        def head_slice(base, h):
            return qkvT[64 * (h % 2):64 * (h % 2) + 64, base + h // 2, :]

        attn_out = pa.tile([128, 2, D], F32, tag="attn_out")  # token-major
        with (
            tc.tile_pool(name="pss", bufs=2, space="PSUM") as pss,
            tc.tile_pool(name="pst", bufs=3, space="PSUM") as pst,
            tc.tile_pool(name="pso", bufs=2, space="PSUM") as pso,
            tc.tile_pool(name="wka", bufs=3) as wka,
        ):
            for h in range(HEADS):
                qT, kT, vT = (head_slice(b, h) for b in (0, 6, 12))
                b0 = HD * (h % 2)
                v_tok = wka.tile([128, 2, HD], F32, tag="v_tok")
                for i, sz in enumerate(NCH):
                    pv = pst.tile([128, 128], F32, tag="ptt")
                    nc.tensor.transpose(pv[:sz, :HD],
                                        vT[:, 128 * i:128 * i + sz],
                                        ident[b0:b0 + HD, b0:b0 + HD])
                    nc.any.tensor_copy(v_tok[:sz, i, :], pv[:sz, :HD])
                attnT = wka.tile([128, 2, N], F32, tag="attnT")
                rcp = wka.tile([128, 2], F32, tag="rcp")
                for i, sz in enumerate(NCH):
                    ps = pss.tile([128, N], F32, tag="sps")
                    nc.tensor.matmul(ps[:sz, :], qT[:, 128 * i:128 * i + sz],
                                     kT[:], start=True, stop=True)
                    m0 = wka.tile([128, 1], F32, tag="sm_m0")
                    nc.vector.tensor_reduce(m0[:sz], ps[:sz, :],
                                            axis=mybir.AxisListType.X,
                                            op=OP.max)
                    m0n = wka.tile([128, 1], F32, tag="sm_m0n")
                    nc.vector.tensor_scalar_mul(m0n[:sz], m0[:sz], -0.125)
                    ex = wka.tile([128, N], F32, tag="sm_ex")
                    ssum = wka.tile([128, 1], F32, tag="sm_sum")
                    nc.scalar.activation(ex[:sz, :], ps[:sz, :], AF.Exp,
                                         bias=m0n[:sz], scale=0.125,
                                         accum_out=ssum[:sz])
                    nc.vector.reciprocal(rcp[:sz, i:i + 1], ssum[:sz])
                    for j, szj in enumerate(NCH):
                        pt = pst.tile([128, 128], F32, tag="ptt")
                        nc.tensor.transpose(pt[:szj, :sz],
                                            ex[:sz, 128 * j:128 * j + szj],
                                            ident[0:sz, 0:sz])
                        nc.any.tensor_copy(
                            attnT[:szj, j, 128 * i:128 * i + sz],
                            pt[:szj, :sz])
                for i, sz in enumerate(NCH):
                    po = pso.tile([128, HD], F32, tag="pso")
                    for j, szj in enumerate(NCH):
                        nc.tensor.matmul(po[:sz, :],
                                         attnT[:szj, j, 128 * i:128 * i + sz],
                                         v_tok[:szj, j, :],
                                         start=(j == 0), stop=(j == 1))
                    nc.any.tensor_scalar(
                        attn_out[:sz, i, 64 * h:64 * (h + 1)], po[:sz, :],
                        rcp[:sz, i:i + 1], None, op0=OP.mult)

        aoT = pa.tile([128, KT, N], F32, tag="aoT")
        with (
            tc.tile_pool(name="psat", bufs=3, space="PSUM") as psat,
            tc.tile_pool(name="psp", bufs=3, space="PSUM") as psp,
        ):
            for ft in range(KT):
                pt = psat.tile([128, N], F32, tag="psatt")
                for i, sz in enumerate(NCH):
                    nc.tensor.transpose(
                        pt[:, 128 * i:128 * i + sz],
                        attn_out[:sz, i, 128 * ft:128 * (ft + 1)],
                        ident[0:sz, 0:sz])
                nc.any.tensor_copy(aoT[:, ft, :], pt[:])
            for ft in range(KT):
                ps = psp.tile([128, N], F32, tag="pspp")
                for k in range(KT):
                    nc.tensor.matmul(ps[:],
                                     projw_sb[:, k, 128 * ft:128 * (ft + 1)],
                                     aoT[:, k, :], start=(k == 0),
                                     stop=(k == KT - 1))
                nc.vector.scalar_tensor_tensor(
                    x1T[:, ft, :], ps[:], cvec["proj_b"][:, ft:ft + 1],
                    xT_sb[:, ft, :], op0=OP.add, op1=OP.add)

    # =====================================================================
    # Phase B: LN2, bounce assembly, AllGather #1
    # =====================================================================
    ag_in = dram.tile([N, D], F32)
    ag_out = dram.tile([T, D], F32, addr_space="Shared")
    agl_in = dram.tile([N, 8], F32)
    agl_out = dram.tile([T, 8], F32, addr_space="Shared")
    with tc.tile_pool(name="p_gate", bufs=1) as pg:
        gatew_sb = pg.tile([128, KT, 8], F32, tag="gatew")
        for k in range(KT):
            nc.sync.dma_start(gatew_sb[:, k, :],
                              ins["gate_w"][128 * k:128 * (k + 1), :])
        xnT = pg.tile([128, KT, N], F32, tag="xnT")
        with (
            tc.tile_pool(name="psln2", bufs=1, space="PSUM") as psln2,
            tc.tile_pool(name="wkln2", bufs=2) as wkln2,
        ):
            layer_norm_fm(x1T, cvec["ln2_g"], cvec["ln2_b"], xnT, psln2,
                          wkln2)

        bounce = pg.tile([128, 2, D], F32, tag="bounce")
        blog = pg.tile([128, 2, 8], F32, tag="blog")
        with tc.tile_pool(name="psb", bufs=2, space="PSUM") as psb:
            for i, sz in enumerate(NCH):
                pl = psb.tile([128, 8], F32, tag="psbl")
                for k in range(KT):
                    nc.tensor.matmul(pl[:sz, :],
                                     xnT[:, k, 128 * i:128 * i + sz],
                                     gatew_sb[:, k, :], start=(k == 0),
                                     stop=(k == KT - 1))
                nc.any.tensor_copy(blog[:sz, i, :], pl[:sz, :])
            for i, sz in enumerate(NCH):
                nc.sync.dma_start(agl_in[128 * i:128 * i + sz, :],
                                  blog[:sz, i, :])
            for i, sz in enumerate(NCH):
                pt = psb.tile([128, D], F32, tag="psbt")
                for ft in range(KT):
                    nc.tensor.transpose(pt[:sz, 128 * ft:128 * (ft + 1)],
                                        xnT[:, ft, 128 * i:128 * i + sz],
                                        ident[:])
                nc.any.tensor_copy(bounce[:sz, i, 0:D], pt[:sz, :])
        for i, sz in enumerate(NCH):
            nc.sync.dma_start(ag_in[128 * i:128 * i + sz, :],
                              bounce[:sz, i, :])
        if "xnT" in dbg:
            for k in range(KT):
                nc.sync.dma_start(dbg["xnT"][128 * k:128 * (k + 1), :],
                                  xnT[:, k, :])
    nc.gpsimd.collective_compute(
        "AllGather", OP.bypass, ins=[agl_in.opt()], outs=[agl_out.opt()],
        replica_groups=[list(range(NCORES))])
    nc.gpsimd.collective_compute(
        "AllGather", OP.bypass, ins=[ag_in.opt()], outs=[ag_out.opt()],
        replica_groups=[list(range(NCORES))])

    # =====================================================================
    # Phase C: routing (replicated on every core)
    # =====================================================================
    lg = sb.tile([128, TW, 8], F32, tag="lg")
    nc.vector.memset(lg[:], 0.0)
    nc.sync.dma_start(
        lg[:, 0:12, :],
        agl_out[0:1536, :].rearrange("(i p) c -> p i c", p=128))
    nc.sync.dma_start(lg[0:32, 12, :], agl_out[1536:T, :])

    gates = sb.tile([128, TW, 8], F32, tag="gates")
    g0wr = sb.tile([128, TW], F32, tag="g0wr")
    for i in range(TW):
        lgi = lg[:, i, :]
        m0 = sbw.tile([128, 1], F32, tag="rt_m0")
        nc.vector.tensor_reduce(m0[:], lgi, axis=mybir.AxisListType.X,
                                op=OP.max)
        m0n = sbw.tile([128, 1], F32, tag="rt_m0n")
        nc.vector.tensor_scalar_mul(m0n[:], m0[:], -1.0)
        mk0 = sbw.tile([128, 8], F32, tag="rt_mk0")
        nc.vector.tensor_scalar(mk0[:], lgi, m0[:], None, op0=OP.is_ge)
        msk = sbw.tile([128, 8], F32, tag="rt_msk")
        nc.vector.scalar_tensor_tensor(msk[:], mk0[:], -1e30, lgi,
                                       op0=OP.mult, op1=OP.add)
        m1 = sbw.tile([128, 1], F32, tag="rt_m1")
        nc.vector.tensor_reduce(m1[:], msk[:], axis=mybir.AxisListType.X,
                                op=OP.max)
        ex = sbw.tile([128, 8], F32, tag="rt_ex")
        nc.scalar.activation(ex[:], lgi, AF.Exp, bias=m0n[:])
        e1 = sbw.tile([128, 1], F32, tag="rt_e1")
        nc.scalar.activation(e1[:], m1[:], AF.Exp, bias=m0n[:])
        den = sbw.tile([128, 1], F32, tag="rt_den")
        nc.vector.tensor_scalar_add(den[:], e1[:], 1.0)
        nc.vector.reciprocal(g0wr[:, i:i + 1], den[:])
        mt = sbw.tile([128, 8], F32, tag="rt_mt")
        nc.vector.tensor_scalar(mt[:], lgi, m1[:], None, op0=OP.is_ge)
        gu = sbw.tile([128, 8], F32, tag="rt_gu")
        nc.vector.tensor_tensor(gu[:], ex[:], mt[:], op=OP.mult)
        nc.vector.tensor_scalar(gates[:, i, :], gu[:], g0wr[:, i:i + 1],
                                None, op0=OP.mult)

    p_rt = tc.alloc_tile_pool(name="p_rt", bufs=1)
    g8_t = p_rt.tile([8, TWPAD], F32, tag="g8")
    mask8_t = p_rt.tile([8, TWPAD], F32, tag="mask8")
    rank8_t = p_rt.tile([8, TWPAD], F32, tag="rank8")
    slotA_t = p_rt.tile([8, TWPAD], F32, tag="slotA")
    s0A_t = p_rt.tile([8, TWPAD], F32, tag="s0A")
    s1A_t = p_rt.tile([8, TWPAD], F32, tag="s1A")
    s0G_t = p_rt.tile([8, TWPAD], F32, tag="s0G")
    mrow_t = p_rt.tile([1, TWPAD], F32, tag="mrow")
    g8 = g8_t[:]
    mask8 = mask8_t[:]
    rank8 = rank8_t[:]
    slotA = slotA_t[:]
    s0A = s0A_t[:]
    s1A = s1A_t[:]
    s0G = s0G_t[:]
    mrow = mrow_t[:]
    with tc.tile_pool(name="psg", bufs=3, space="PSUM") as psg:
        for i in range(TW):
            pt = psg.tile([8, 128], F32, tag="psgt")
            nc.tensor.transpose(pt[:], gates[:, i, :], ident[:])
            nc.any.tensor_copy(g8[:, 128 * i:128 * (i + 1)], pt[:])

    nc.vector.tensor_scalar(mask8, g8, 0.0, None, op0=OP.is_gt)
    nc.vector.tensor_tensor_scan(rank8, mask8, mask8, initial=0.0,
                                 op0=OP.add, op1=OP.bypass)
    eoff_i = sb.tile([8, 1], I32, tag="eoff_i")
    nc.gpsimd.iota(eoff_i[:], pattern=[[1, 1]], base=-1, channel_multiplier=C)
    eoff = sb.tile([8, 1], F32, tag="eoff")
    nc.vector.tensor_copy(eoff[:], eoff_i[:])
    # slotA = rank8 - 1 + C*e ; sel0 = g8 >= .5 (fused below)
    nc.vector.tensor_scalar(slotA, rank8, eoff[:], None, op0=OP.add)
    nc.vector.scalar_tensor_tensor(s0A, g8, 0.5, slotA,
                                   op0=OP.is_ge, op1=OP.mult)
    nc.vector.tensor_tensor(s1A, mask8, slotA, op=OP.mult)
    nc.vector.tensor_tensor(s1A, s1A, s0A, op=OP.subtract)
    nc.vector.scalar_tensor_tensor(s0G, g8, 0.5, g8,
                                   op0=OP.is_ge, op1=OP.mult)

    # my-token window [tok0, tok0+196) via dynamic rhs slice
    treg = nc.tensor.alloc_register("tok0reg")
    nc.tensor.reg_load(treg, tok0_sb[0:1, 0:1])
    toff = nc.tensor.snap(treg, donate=True, min_val=0,
                          max_val=N * (NCORES - 1))

    c0my16 = sb.tile([1, N], I16, tag="c0my16")
    c1my16 = sb.tile([1, N], I16, tag="c1my16")
    g0my = sb.tile([1, N], F32, tag="g0my")
    with tc.tile_pool(name="psc", bufs=3, space="PSUM") as psc:
        for srcv, base, dst in ((s0A, 0, c0my16), (s1A, 0, c1my16),
                                (s0G, 0, g0my)):
            p = psc.tile([1, N], F32, tag="pscc")
            nc.tensor.matmul(p[:], ones_col[base:base + 8, 0:1],
                             srcv[:, bass.ds(toff, N)],
                             start=True, stop=True)
            nc.any.tensor_copy(dst[:], p[:])

    # combine idx: [c0 | pad | c1 | pad] -> dram[512] -> wrapped16 -> 128p
    c01 = sb.tile([1, 512], I16, tag="c01")
    nc.vector.memset(c01[:], 0)
    nc.vector.tensor_copy(c01[:, 0:N], c0my16[:])
    nc.vector.tensor_copy(c01[:, 256:256 + N], c1my16[:])
    cidx_d = dram.tile([1, 512], I16)
    nc.sync.dma_start(cidx_d[:], c01[:])
    cidx16 = sb.tile([16, 32], I16, tag="cidx16")
    nc.sync.dma_start(cidx16[:], cidx_d[:].rearrange("1 (f p) -> p f", p=16))
    cidx128 = sb.tile([128, 32], I16, tag="cidx128")
    _repl16(nc, cidx128, cidx16)

    # g0 window -> wrapped [128, 2]
    g0_d = dram.tile([1, N], F32)
    nc.sync.dma_start(g0_d[:], g0my[:])
    g0w2 = sb.tile([128, 2], F32, tag="g0w2")
    nc.vector.memset(g0w2[:], 0.0)
    nc.sync.dma_start(g0w2[:, 0:1], g0_d[0:1, 0:128].rearrange("1 p -> p 1"))
    nc.sync.dma_start(g0w2[0:N - 128, 1:2],
                      g0_d[0:1, 128:N].rearrange("1 p -> p 1"))

    # dispatch idx: my expert row of mask8 -> compacted token list
    with tc.tile_pool(name="psm", bufs=2, space="PSUM") as psm:
        for c0, cw in ((0, 512), (512, 512), (1024, 512), (1536, 128)):
            p = psm.tile([1, cw], F32, tag="psmm")
            nc.tensor.matmul(p[:], eoh_sb[0:8, 0:1], mask8[:, c0:c0 + cw],
                             start=True, stop=True)
            nc.any.tensor_copy(mrow[0:1, c0:c0 + cw], p[:])
    mrow_d = dram.tile([1, TWPAD], F32)
    nc.sync.dma_start(mrow_d[:], mrow)
    m16 = sb.tile([16, T // 16], F32, tag="m16")
    nc.sync.dma_start(m16[:],
                      mrow_d[0:1, 0:T].rearrange("1 (f p) -> p f", p=16))
    iot = sb.tile([16, T // 16], I32, tag="iot")
    nc.gpsimd.iota(iot[:], pattern=[[16, T // 16]], base=0,
                   channel_multiplier=1)
    iotf = sb.tile([16, T // 16], F32, tag="iotf")
    nc.vector.tensor_copy(iotf[:], iot[:])
    cand = sb.tile([16, T // 16], F32, tag="cand")
    nc.vector.tensor_scalar_add(cand[:], iotf[:], 1.0)
    nc.vector.tensor_tensor(cand[:], cand[:], m16[:], op=OP.mult)
    nc.vector.tensor_scalar_add(cand[:], cand[:], -1.0)
    nfound = sb.tile([1, 1], U32, tag="nfound")
    comp = sb.tile([16, C // 16], F32, tag="comp")
    nc.gpsimd.sparse_gather(comp[:], cand[:], num_found=nfound[:])
    nc.vector.tensor_scalar(comp[:], comp[:], float(T - 1), 0.0,
                            op0=OP.min, op1=OP.max)
    didx16 = sb.tile([16, C // 16], I16, tag="didx16")
    nc.vector.tensor_copy(didx16[:], comp[:])
    didx128 = sb.tile([128, C // 16], I16, tag="didx128")
    _repl16(nc, didx128, didx16)

    if "rank8" in dbg:
        nc.sync.dma_start(dbg["rank8"][:], rank8)
    p_rt.release()

    # x1 token-major + b2 (fills the PE gap during routing/collectives)
    with tc.tile_pool(name="psr", bufs=2, space="PSUM") as psr:
        for i, sz in enumerate(NCH):
            pt = psr.tile([128, D], F32, tag="psrt")
            for ft in range(KT):
                nc.tensor.transpose(pt[:sz, 128 * ft:128 * (ft + 1)],
                                    x1T[:, ft, 128 * i:128 * i + sz],
                                    ident[:])
            nc.vector.tensor_tensor(x1tok[:sz, i, :], pt[:sz, :],
                                    b2_b[:sz, :], op=OP.add)

    # =====================================================================
    # Phase D: dispatch gather, expert FFN, AllGather #2
    # =====================================================================
    y_d = dram.tile([C, D], F32)
    p_hT = tc.alloc_tile_pool(name="p_hT", bufs=1)
    with tc.tile_pool(name="p_w1", bufs=1) as pw1:
        w1_sb = pw1.tile([128, KT, HID], F32R, tag="w1")
        for k in range(KT):
            nc.sync.dma_start(w1_sb[:, k, :],
                              ins["w1"][128 * k:128 * (k + 1), :]
                              .bitcast(F32R))
        XgT = pw1.tile([128, KT, C], F32R, tag="XgT")
        with tc.tile_pool(name="p_xg", bufs=1) as pxg:
            Xg = pxg.tile([128, CM, D], F32, tag="Xg")
            nc.gpsimd.dma_gather(Xg[:], ag_out[:], didx128[:],
                                 num_idxs=C, num_idxs_reg=C, elem_size=D)
            with tc.tile_pool(name="psx", bufs=3, space="PSUM") as psx:
                for ct in range(CM):
                    for ft in range(KT):
                        pt = psx.tile([128, 128], F32, tag="psxt")
                        nc.tensor.transpose(
                            pt[:], Xg[:, ct, 128 * ft:128 * (ft + 1)],
                            ident[:])
                        nc.any.tensor_copy(
                            XgT[:, ft, 128 * ct:128 * (ct + 1)], pt[:])

        hT = p_hT.tile([128, HM, C], F32R, tag="hT")
        with tc.tile_pool(name="ps1", bufs=3, space="PSUM") as ps1:
            for m in range(HM):
                ph = ps1.tile([128, C], F32, tag="ps1t")
                for k in range(KT):
                    nc.tensor.matmul(ph[:],
                                     w1_sb[:, k, 128 * m:128 * (m + 1)],
                                     XgT[:, k, :], start=(k == 0),
                                     stop=(k == KT - 1))
                nc.scalar.activation(hT[:, m, :], ph[:], AF.Gelu,
                                     bias=cvec["b1"][:, m:m + 1])

    with (
        tc.tile_pool(name="p_y", bufs=1) as py,
        tc.tile_pool(name="ps4", bufs=1, space="PSUM") as ps4,
        tc.tile_pool(name="w2p", bufs=3) as w2p,
    ):
        y_sb = py.tile([128, CM, D], F32, tag="y_sb")
        yps = {}
        for mc in range(CM):
            for nb in range(2):
                ypst = ps4.tile([128, 384], F32, tag=f"y{mc}{nb}")
                yps[(mc, nb)] = ypst
        for k in range(HM):
            w2t = w2p.tile([128, D], F32R, tag="w2t")
            nc.sync.dma_start(w2t[:],
                              ins["w2"][128 * k:128 * (k + 1), :]
                              .bitcast(F32R))
            for mc in range(CM):
                for nb in range(2):
                    nc.tensor.matmul(
                        yps[(mc, nb)][:],
                        hT[:, k, 128 * mc:128 * (mc + 1)],
                        w2t[:, 384 * nb:384 * (nb + 1)],
                        start=(k == 0), stop=(k == HM - 1))
        for mc in range(CM):
            for nb in range(2):
                nc.any.tensor_copy(y_sb[:, mc, 384 * nb:384 * (nb + 1)],
                                   yps[(mc, nb)][:])
        nc.sync.dma_start(y_d[:].rearrange("(i p) d -> p i d", p=128),
                          y_sb[:])
        if "ysb" in dbg:
            nc.sync.dma_start(
                dbg["ysb"][:].rearrange("(i p) d -> p i d", p=128), y_sb[:])

    p_hT.release()
    yall = dram.tile([NCORES * C, D], F32, addr_space="Shared")
    nc.gpsimd.collective_compute(
        "AllGather", OP.bypass, ins=[y_d.opt()], outs=[yall.opt()],
        replica_groups=[list(range(NCORES))])

    # =====================================================================
    # Phase E: combine gather + weighted sum + output
    # =====================================================================
    with tc.tile_pool(name="p_e", bufs=1) as pe:
        g_tok = pe.tile([128, 4, D], F32, tag="g_tok")
        nc.gpsimd.dma_gather(g_tok[:], yall[:], cidx128[:],
                             num_idxs=512, num_idxs_reg=512, elem_size=D)
        for i, sz in enumerate(NCH):
            dtile = sbw.tile([128, D], F32, tag="cmb_d")
            nc.vector.tensor_tensor(dtile[:sz, :], g_tok[:sz, i, :],
                                    g_tok[:sz, i + 2, :], op=OP.subtract)
            t2 = sbw.tile([128, D], F32, tag="cmb_t2")
            nc.vector.scalar_tensor_tensor(t2[:sz, :], dtile[:sz, :],
                                           g0w2[:sz, i:i + 1],
                                           g_tok[:sz, i + 2, :],
                                           op0=OP.mult, op1=OP.add)
            ocmb = sbw.tile([128, D], F32, tag="cmb_o")
            nc.vector.tensor_tensor(ocmb[:sz, :], t2[:sz, :],
                                    x1tok[:sz, i, :], op=OP.add)
            nc.sync.dma_start(out[128 * i:128 * i + sz, :], ocmb[:sz, :])

    # remaining debug taps
    if "gates" in dbg:
        for i in range(TW):
            nc.sync.dma_start(dbg["gates"][0:128, 8 * i:8 * (i + 1)],
                              gates[:, i, :])
    if "cidx" in dbg:
        nc.sync.dma_start(dbg["cidx"][:], cidx128[0:16, :])
    if "didx" in dbg:
        nc.sync.dma_start(dbg["didx"][:], didx128[0:16, :])
    if "g0w2" in dbg:
        nc.sync.dma_start(dbg["g0w2"][:], g0w2[:])
    if "x1T" in dbg:
        for k in range(KT):
            nc.sync.dma_start(dbg["x1T"][128 * k:128 * (k + 1), :],
                              x1T[:, k, :])

    sbw.release()
    sb.release()
    dram.release()


# ---------------------------------------------------------------------------
# host side
# ---------------------------------------------------------------------------

DBG_SPECS = {
    "xnT": (D, N), "gates": (128, TW * 8), "rank8": (8, TWPAD),
    "cidx": (16, 32), "didx": (16, C // 16), "g0w2": (128, 2),
    "ysb": (C, D), "x1T": (D, N),
}

_PROGRAM_CACHE = {}


def get_program(debug=False):
    key = bool(debug)
    if key in _PROGRAM_CACHE:
        return _PROGRAM_CACHE[key]
    nc = bacc.Bacc("TRN2", target_bir_lowering=False, debug=False,
                   enable_asserts=True, num_devices=NCORES)
    ins = {}

    def din(name, shape, dtype=F32):
        ins[name] = nc.dram_tensor(name, list(shape), dtype,
                                   kind="ExternalInput").ap()

    din("xT", (D, N))
    din("qkv_w", (D, 3 * D))
    din("qkv_b", (128, 18))
    din("proj_w", (D, D))
    din("proj_b", (128, KT))
    din("ln1_g", (128, KT))
    din("ln1_b", (128, KT))
    din("ln2_g", (128, KT))
    din("ln2_b", (128, KT))
    din("gate_w", (D, E))
    din("w1", (D, HID))
    din("b1", (128, HM))
    din("w2", (HID, D))
    din("b2", (1, D))
    din("tok0", (1, 1), I32)
    din("eoh", (128, 1))

    outs = {"out": nc.dram_tensor("out", [N, D], F32,
                                  kind="ExternalOutput").ap()}
    if debug:
        outs["dbg"] = {
            nm: nc.dram_tensor(f"dbg_{nm}", list(shp),
                               I16 if nm in ("cidx", "didx") else F32,
                               kind="ExternalOutput").ap()
            for nm, shp in DBG_SPECS.items()
        }

    with tile.TileContext(nc) as tc:
        build_block(tc, outs, ins)
    nc.compile()
    _PROGRAM_CACHE[key] = nc
    return nc


def round_f32r(x):
    """Round fp32 to fp32r (8e11m) with round-to-nearest-even."""
    b = np.ascontiguousarray(x, np.float32).view(np.uint32).copy()
    lsb = (b >> np.uint32(12)) & np.uint32(1)
    r = b + np.uint32(0x7FF) + lsb
    return (r & np.uint32(0xFFFFF000)).view(np.float32)


def prep_in_maps(inputs):
    f = {k: np.ascontiguousarray(np.asarray(v), dtype=np.float32)
         for k, v in inputs.items()}
    share = {
        "qkv_w": f["qkv_w"],
        "qkv_b": f["qkv_b"].reshape(18, 128).T.copy(),
        "proj_w": f["proj_w"],
        "proj_b": f["proj_b"].reshape(KT, 128).T.copy(),
        "ln1_g": f["ln1_g"].reshape(KT, 128).T.copy(),
        "ln1_b": f["ln1_b"].reshape(KT, 128).T.copy(),
        "ln2_g": f["ln2_g"].reshape(KT, 128).T.copy(),
        "ln2_b": f["ln2_b"].reshape(KT, 128).T.copy(),
        "gate_w": f["gate_w"],
    }
    in_maps = []
    for o in range(NCORES):
        m = dict(share)
        m["xT"] = f["x"][o].T.copy()
        m["w1"] = round_f32r(f["w1"][o])
        m["b1"] = f["b1"][o].reshape(HM, 128).T.copy()
        m["w2"] = round_f32r(f["w2"][o])
        m["b2"] = f["b2"][o].reshape(1, D).copy()
        m["tok0"] = np.array([[o * N]], np.int32)
        eoh = np.zeros((8, 1), np.float32)
        eoh[o, 0] = 1.0
        m["eoh"] = np.tile(eoh, (16, 1))
        in_maps.append(m)
    return in_maps


def kernel(**inputs):
    nc = get_program(debug=False)
    in_maps = prep_in_maps(inputs)
    res = bass_utils.run_bass_kernel_spmd(
        nc, in_maps, core_ids=list(range(NCORES)), trace=False)
    out = np.stack([r["out"] for r in res.results], axis=0)
    return out.astype(np.float32)


# revision 20
# speedup vs baseline: 1.3412x; 1.0823x over previous
"""MoE transformer block (attention + top-2 routed 8-expert FFN) on 8 TRN2
NeuronCores.

Sharding: data-parallel attention (1 image of 196 tokens per core) +
expert-parallel MoE (1 expert per core). Token dispatch/combine via
AllGather + dma_gather, with slot ranks computed on-device by a cumsum
scan over the routing mask.

Self-contained: hardcodes all shapes; imports only concourse (already on
PYTHONPATH in the runtime image).
"""
import os
import sys
for _p in ("/opt/trn_rl_repo", "/root/.axon_site/_ro/trn_rl_repo"):
    if _p not in sys.path:
        sys.path.append(_p)

import numpy as np
import concourse.bass as bass
import concourse.bacc as bacc
import concourse.mybir as mybir
import concourse.tile as tile
from concourse import masks
from concourse import bass_utils

F32 = mybir.dt.float32
F32R = mybir.dt.float32r
BF16 = mybir.dt.bfloat16
I16 = mybir.dt.int16
I32 = mybir.dt.int32
U32 = mybir.dt.uint32
AF = mybir.ActivationFunctionType
OP = mybir.AluOpType

NCORES = 8
B, N, D = 8, 196, 768
HEADS, HD = 12, 64
E, HID = 8, 3072
T = B * N                  # 1568
C = 512                    # expert capacity (max observed count is 424)
KT = D // 128              # 6 feature k-tiles
HM = HID // 128            # 24 hidden tiles
CM = C // 128              # 4 capacity tiles
LN_EPS = 1e-5
ROWB = 832                 # AG row: 768 xn + 8 logits + pad to 256B multiple
TW = 13                    # ceil(T/128) wrapped token tiles
TWPAD = TW * 128           # 1664
NCH = [128, N - 128]       # valid partitions per token chunk (128 + 68)


def _repl16(nc, dst, src16):
    """Replicate a [16, f] tile across all 128 partitions of dst."""
    nc.vector.memset(dst[:], 0)
    nc.vector.tensor_copy(dst[0:16, :], src16[:])
    nc.vector.stream_shuffle(dst[0:32, :], dst[0:32, :],
                             mask=[i % 16 for i in range(32)])
    nc.vector.tensor_copy(dst[32:64, :], dst[0:32, :])
    nc.vector.tensor_copy(dst[64:128, :], dst[0:64, :])


def build_block(tc, outs, ins):
    nc = tc.nc
    out = outs["out"]          # [196, 768] per-core output
    dbg = outs.get("dbg", {})  # optional {name: AP} debug outputs

    sb = tc.alloc_tile_pool(name="sb", bufs=1)      # small persistents
    sbw = tc.alloc_tile_pool(name="sbw", bufs=3)    # loop workspace
    dram = tc.alloc_tile_pool(name="dram", bufs=1, space="DRAM")

    ident = sb.tile([128, 128], F32)
    masks.make_identity(nc, ident[:])
    ident_bf = sb.tile([128, 128], BF16)
    nc.vector.tensor_copy(ident_bf[:], ident[:])
    ones_col = sb.tile([128, 1], F32)
    nc.vector.memset(ones_col[:], 1.0)
    ones8 = sb.tile([8, 1], F32)
    nc.vector.memset(ones8[:], 1.0)
    eps_sb = sb.tile([1, 1], F32)
    nc.vector.memset(eps_sb[:], LN_EPS)

    cvec = {}
    for nm, w in (("qkv_b", 18), ("proj_b", KT), ("ln1_g", KT), ("ln1_b", KT),
                  ("ln2_g", KT), ("ln2_b", KT), ("b1", HM)):
        t = sb.tile([128, w], F32, tag=nm)
        nc.sync.dma_start(t[:], ins[nm][:])
        cvec[nm] = t
    b2_row = sb.tile([1, D], F32, tag="b2r")
    nc.sync.dma_start(b2_row[:], ins["b2"][:])
    b2_b = sb.tile([128, D], F32, tag="b2b")
    nc.gpsimd.partition_broadcast(b2_b[:], b2_row[:])

    tok0_sb = sb.tile([1, 1], I32, tag="tok0")
    nc.sync.dma_start(tok0_sb[:], ins["tok0"][:])
    eoh_sb = sb.tile([128, 1], F32, tag="eoh")
    nc.sync.dma_start(eoh_sb[:], ins["eoh"][:])

    # medium-lived persistents
    x1T = sb.tile([128, KT, N], F32, tag="x1T")
    x1tok = sb.tile([128, 2, D], F32, tag="x1tok")

    # ---- feature-major LayerNorm via matmul-ones stats -------------------
    def layer_norm_fm(src, g_t, b_t, dst, psln, wk):
        s1 = psln.tile([1, N], F32, tag="lnp1")
        for k in range(KT):
            nc.tensor.matmul(s1[:], ones_col[:], src[:, k, :],
                             start=(k == 0), stop=(k == KT - 1))
        s2 = psln.tile([1, N], F32, tag="lnp2")
        for k in range(KT):
            sq = wk.tile([128, N], F32, tag="ln_sq")
            nc.scalar.activation(sq[:], src[:, k, :], AF.Square)
            nc.tensor.matmul(s2[:], ones_col[:], sq[:],
                             start=(k == 0), stop=(k == KT - 1))
        mu = wk.tile([1, N], F32, tag="ln_mu")
        nc.vector.tensor_scalar_mul(mu[:], s1[:], 1.0 / D)
        nmu2 = wk.tile([1, N], F32, tag="ln_nmu2")
        nc.vector.scalar_tensor_tensor(nmu2[:], mu[:], -1.0, mu[:],
                                       op0=OP.mult, op1=OP.mult)
        var = wk.tile([1, N], F32, tag="ln_var")
        nc.vector.scalar_tensor_tensor(var[:], s2[:], 1.0 / D, nmu2[:],
                                       op0=OP.mult, op1=OP.add)
        sd = wk.tile([1, N], F32, tag="ln_sd")
        nc.scalar.activation(sd[:], var[:], AF.Sqrt, bias=eps_sb[:])
        rstd = wk.tile([1, N], F32, tag="ln_rstd")
        nc.vector.reciprocal(rstd[:], sd[:])
        mu_b = wk.tile([128, N], F32, tag="ln_mub")
        nc.gpsimd.partition_broadcast(mu_b[:], mu[:])
        rstd_b = wk.tile([128, N], F32, tag="ln_rstdb")
        nc.gpsimd.partition_broadcast(rstd_b[:], rstd[:])
        for k in range(KT):
            tmp = wk.tile([128, N], F32, tag="ln_tmp")
            nc.vector.tensor_tensor(tmp[:], src[:, k, :], mu_b[:],
                                    op=OP.subtract)
            nc.vector.tensor_tensor(tmp[:], tmp[:], rstd_b[:], op=OP.mult)
            nc.any.tensor_scalar(dst[:, k, :], tmp[:], g_t[:, k:k + 1],
                                 b_t[:, k:k + 1], op0=OP.mult, op1=OP.add)

    # =====================================================================
    # Phase A: attention (fp32, feature-major)
    # =====================================================================
    with (
        tc.tile_pool(name="p_attn", bufs=1) as pa,
        tc.tile_pool(name="p_qkvw", bufs=1) as pqw,
    ):
        xT_sb = pa.tile([128, KT, N], F32, tag="xT")
        projw_sb = pa.tile([128, KT, D], F32, tag="projw")
        qkvw_sb = pqw.tile([128, KT, 2304], F32, tag="qkvw")
        for k in range(KT):
            nc.sync.dma_start(xT_sb[:, k, :],
                              ins["xT"][128 * k:128 * (k + 1), :])
        for k in range(KT):
            nc.sync.dma_start(qkvw_sb[:, k, :],
                              ins["qkv_w"][128 * k:128 * (k + 1), :])
        for k in range(KT):
            nc.sync.dma_start(projw_sb[:, k, :],
                              ins["proj_w"][128 * k:128 * (k + 1), :])

        xn1 = pa.tile([128, KT, N], F32, tag="xn1")
        with (
            tc.tile_pool(name="psln", bufs=1, space="PSUM") as psln,
            tc.tile_pool(name="wkln1", bufs=2) as wkln1,
        ):
            layer_norm_fm(xT_sb, cvec["ln1_g"], cvec["ln1_b"], xn1, psln,
                          wkln1)

        qkvT = pa.tile([128, 18, N], F32, tag="qkvT")
        with tc.tile_pool(name="psqkv", bufs=3, space="PSUM") as psqkv:
            for m in range(18):
                ps = psqkv.tile([128, N], F32, tag="qkvps")
                for k in range(KT):
                    nc.tensor.matmul(
                        ps[:], qkvw_sb[:, k, 128 * m:128 * (m + 1)],
                        xn1[:, k, :], start=(k == 0), stop=(k == KT - 1))
                nc.any.tensor_scalar(qkvT[:, m, :], ps[:],
                                     cvec["qkv_b"][:, m:m + 1], None,
                                     op0=OP.add)

        def head_slice(base, h):
            return qkvT[64 * (h % 2):64 * (h % 2) + 64, base + h // 2, :]

        attn_out = pa.tile([128, 2, D], F32, tag="attn_out")  # token-major
        with (
            tc.tile_pool(name="pss", bufs=2, space="PSUM") as pss,
            tc.tile_pool(name="pst", bufs=3, space="PSUM") as pst,
            tc.tile_pool(name="pso", bufs=2, space="PSUM") as pso,
            tc.tile_pool(name="wka", bufs=3) as wka,
        ):
            for h in range(HEADS):
                qT, kT, vT = (head_slice(b, h) for b in (0, 6, 12))
                b0 = HD * (h % 2)
                v_tok = wka.tile([128, 2, HD], F32, tag="v_tok")
                for i, sz in enumerate(NCH):
                    pv = pst.tile([128, 128], F32, tag="ptt")
                    nc.tensor.transpose(pv[:sz, :HD],
                                        vT[:, 128 * i:128 * i + sz],
                                        ident[b0:b0 + HD, b0:b0 + HD])
                    nc.any.tensor_copy(v_tok[:sz, i, :], pv[:sz, :HD])
                attnT = wka.tile([128, 2, N], F32, tag="attnT")
                rcp = wka.tile([128, 2], F32, tag="rcp")
                for i, sz in enumerate(NCH):
                    ps = pss.tile([128, N], F32, tag="sps")
                    nc.tensor.matmul(ps[:sz, :], qT[:, 128 * i:128 * i + sz],
                                     kT[:], start=True, stop=True)
                    m0 = wka.tile([128, 1], F32, tag="sm_m0")
                    nc.vector.tensor_reduce(m0[:sz], ps[:sz, :],
                                            axis=mybir.AxisListType.X,
                                            op=OP.max)
                    m0n = wka.tile([128, 1], F32, tag="sm_m0n")
                    nc.vector.tensor_scalar_mul(m0n[:sz], m0[:sz], -0.125)
                    ex = wka.tile([128, N], F32, tag="sm_ex")
                    ssum = wka.tile([128, 1], F32, tag="sm_sum")
                    nc.scalar.activation(ex[:sz, :], ps[:sz, :], AF.Exp,
                                         bias=m0n[:sz], scale=0.125,
                                         accum_out=ssum[:sz])
                    nc.vector.reciprocal(rcp[:sz, i:i + 1], ssum[:sz])
                    for j, szj in enumerate(NCH):
                        pt = pst.tile([128, 128], F32, tag="ptt")
                        nc.tensor.transpose(pt[:szj, :sz],
                                            ex[:sz, 128 * j:128 * j + szj],
                                            ident[0:sz, 0:sz])
                        nc.any.tensor_copy(
                            attnT[:szj, j, 128 * i:128 * i + sz],
                            pt[:szj, :sz])
                for i, sz in enumerate(NCH):
                    po = pso.tile([128, HD], F32, tag="pso")
                    for j, szj in enumerate(NCH):
                        nc.tensor.matmul(po[:sz, :],
                                         attnT[:szj, j, 128 * i:128 * i + sz],
                                         v_tok[:szj, j, :],
                                         start=(j == 0), stop=(j == 1))
                    nc.any.tensor_scalar(
                        attn_out[:sz, i, 64 * h:64 * (h + 1)], po[:sz, :],
                        rcp[:sz, i:i + 1], None, op0=OP.mult)

        aoT = pa.tile([128, KT, N], F32, tag="aoT")
        with (
            tc.tile_pool(name="psat", bufs=3, space="PSUM") as psat,
            tc.tile_pool(name="psp", bufs=3, space="PSUM") as psp,
        ):
            for ft in range(KT):
                pt = psat.tile([128, N], F32, tag="psatt")
                for i, sz in enumerate(NCH):
                    nc.tensor.transpose(
                        pt[:, 128 * i:128 * i + sz],
                        attn_out[:sz, i, 128 * ft:128 * (ft + 1)],
                        ident[0:sz, 0:sz])
                nc.any.tensor_copy(aoT[:, ft, :], pt[:])
            for ft in range(KT):
                ps = psp.tile([128, N], F32, tag="pspp")
                for k in range(KT):
                    nc.tensor.matmul(ps[:],
                                     projw_sb[:, k, 128 * ft:128 * (ft + 1)],
                                     aoT[:, k, :], start=(k == 0),
                                     stop=(k == KT - 1))
                nc.vector.scalar_tensor_tensor(
                    x1T[:, ft, :], ps[:], cvec["proj_b"][:, ft:ft + 1],
                    xT_sb[:, ft, :], op0=OP.add, op1=OP.add)

    # =====================================================================
    # Phase B: LN2, bounce assembly, AllGather #1
    # =====================================================================
    ag_in = dram.tile([N, D], BF16)
    ag_out = dram.tile([T, D], BF16, addr_space="Shared")
    agl_in = dram.tile([N, 8], F32)
    agl_out = dram.tile([T, 8], F32, addr_space="Shared")
    with tc.tile_pool(name="p_gate", bufs=1) as pg:
        gatew_sb = pg.tile([128, KT, 8], F32, tag="gatew")
        for k in range(KT):
            nc.sync.dma_start(gatew_sb[:, k, :],
                              ins["gate_w"][128 * k:128 * (k + 1), :])
        xnT = pg.tile([128, KT, N], F32, tag="xnT")
        with (
            tc.tile_pool(name="psln2", bufs=1, space="PSUM") as psln2,
            tc.tile_pool(name="wkln2", bufs=2) as wkln2,
        ):
            layer_norm_fm(x1T, cvec["ln2_g"], cvec["ln2_b"], xnT, psln2,
                          wkln2)

        bounce = pg.tile([128, 2, D], BF16, tag="bounce")
        blog = pg.tile([128, 2, 8], F32, tag="blog")
        with tc.tile_pool(name="psb", bufs=2, space="PSUM") as psb:
            for i, sz in enumerate(NCH):
                pl = psb.tile([128, 8], F32, tag="psbl")
                for k in range(KT):
                    nc.tensor.matmul(pl[:sz, :],
                                     xnT[:, k, 128 * i:128 * i + sz],
                                     gatew_sb[:, k, :], start=(k == 0),
                                     stop=(k == KT - 1))
                nc.any.tensor_copy(blog[:sz, i, :], pl[:sz, :])
            for i, sz in enumerate(NCH):
                nc.sync.dma_start(agl_in[128 * i:128 * i + sz, :],
                                  blog[:sz, i, :])
            for i, sz in enumerate(NCH):
                pt = psb.tile([128, D], F32, tag="psbt")
                for ft in range(KT):
                    nc.tensor.transpose(pt[:sz, 128 * ft:128 * (ft + 1)],
                                        xnT[:, ft, 128 * i:128 * i + sz],
                                        ident[:])
                nc.any.tensor_copy(bounce[:sz, i, 0:D], pt[:sz, :])
        for i, sz in enumerate(NCH):
            nc.sync.dma_start(ag_in[128 * i:128 * i + sz, :],
                              bounce[:sz, i, :])
        if "xnT" in dbg:
            for k in range(KT):
                nc.sync.dma_start(dbg["xnT"][128 * k:128 * (k + 1), :],
                                  xnT[:, k, :])
    nc.gpsimd.collective_compute(
        "AllGather", OP.bypass, ins=[agl_in.opt()], outs=[agl_out.opt()],
        replica_groups=[list(range(NCORES))])
    nc.gpsimd.collective_compute(
        "AllGather", OP.bypass, ins=[ag_in.opt()], outs=[ag_out.opt()],
        replica_groups=[list(range(NCORES))])

    # =====================================================================
    # Phase C: routing (replicated on every core)
    # =====================================================================
    lg = sb.tile([128, TW, 8], F32, tag="lg")
    nc.vector.memset(lg[:], 0.0)
    nc.sync.dma_start(
        lg[:, 0:12, :],
        agl_out[0:1536, :].rearrange("(i p) c -> p i c", p=128))
    nc.sync.dma_start(lg[0:32, 12, :], agl_out[1536:T, :])

    gates = sb.tile([128, TW, 8], F32, tag="gates")
    g0wr = sb.tile([128, TW], F32, tag="g0wr")
    m0 = sb.tile([128, TW], F32, tag="rt_m0")
    nc.vector.tensor_reduce(m0[:].rearrange("p (i o) -> p i o", o=1), lg[:],
                            axis=mybir.AxisListType.X, op=OP.max)
    m0v = m0[:].rearrange("p (i o) -> p i o", o=1).to_broadcast([128, TW, 8])
    mk0 = sb.tile([128, TW, 8], F32, tag="rt_mk0")
    nc.vector.tensor_tensor(mk0[:], lg[:], m0v, op=OP.is_ge)
    msk = sb.tile([128, TW, 8], F32, tag="rt_msk")
    nc.vector.scalar_tensor_tensor(msk[:], mk0[:], -1e30, lg[:],
                                   op0=OP.mult, op1=OP.add)
    m1 = sb.tile([128, TW], F32, tag="rt_m1")
    nc.vector.tensor_reduce(m1[:].rearrange("p (i o) -> p i o", o=1), msk[:],
                            axis=mybir.AxisListType.X, op=OP.max)
    m1v = m1[:].rearrange("p (i o) -> p i o", o=1).to_broadcast([128, TW, 8])
    exd = sb.tile([128, TW, 8], F32, tag="rt_exd")
    nc.vector.tensor_tensor(exd[:], lg[:], m0v, op=OP.subtract)
    ex = sb.tile([128, TW, 8], F32, tag="rt_ex")
    nc.scalar.activation(ex[:], exd[:], AF.Exp)
    e1d = sb.tile([128, TW], F32, tag="rt_e1d")
    nc.vector.tensor_tensor(e1d[:], m1[:], m0[:], op=OP.subtract)
    e1 = sb.tile([128, TW], F32, tag="rt_e1")
    nc.scalar.activation(e1[:], e1d[:], AF.Exp)
    den = sb.tile([128, TW], F32, tag="rt_den")
    nc.vector.tensor_scalar_add(den[:], e1[:], 1.0)
    nc.vector.reciprocal(g0wr[:], den[:])
    g0v = g0wr[:].rearrange("p (i o) -> p i o", o=1).to_broadcast([128, TW, 8])
    mt = sb.tile([128, TW, 8], F32, tag="rt_mt")
    nc.vector.tensor_tensor(mt[:], lg[:], m1v, op=OP.is_ge)
    gu = sb.tile([128, TW, 8], F32, tag="rt_gu")
    nc.vector.tensor_tensor(gu[:], ex[:], mt[:], op=OP.mult)
    nc.vector.tensor_tensor(gates[:], gu[:], g0v, op=OP.mult)

    p_rt = tc.alloc_tile_pool(name="p_rt", bufs=1)
    g8_t = p_rt.tile([8, TWPAD], F32, tag="g8")
    mask8_t = p_rt.tile([8, TWPAD], F32, tag="mask8")
    rank8_t = p_rt.tile([8, TWPAD], F32, tag="rank8")
    slotA_t = p_rt.tile([8, TWPAD], F32, tag="slotA")
    s0A_t = p_rt.tile([8, TWPAD], F32, tag="s0A")
    s1A_t = p_rt.tile([8, TWPAD], F32, tag="s1A")
    s0G_t = p_rt.tile([8, TWPAD], F32, tag="s0G")
    mrow_t = p_rt.tile([1, TWPAD], F32, tag="mrow")
    g8 = g8_t[:]
    mask8 = mask8_t[:]
    rank8 = rank8_t[:]
    slotA = slotA_t[:]
    s0A = s0A_t[:]
    s1A = s1A_t[:]
    s0G = s0G_t[:]
    mrow = mrow_t[:]
    with tc.tile_pool(name="psg", bufs=3, space="PSUM") as psg:
        for i in range(TW):
            pt = psg.tile([8, 128], F32, tag="psgt")
            nc.tensor.transpose(pt[:], gates[:, i, :], ident[:])
            nc.any.tensor_copy(g8[:, 128 * i:128 * (i + 1)], pt[:])

    nc.vector.tensor_scalar(mask8, g8, 0.0, None, op0=OP.is_gt)
    nc.vector.tensor_tensor_scan(rank8, mask8, mask8, initial=0.0,
                                 op0=OP.add, op1=OP.bypass)
    eoff_i = sb.tile([8, 1], I32, tag="eoff_i")
    nc.gpsimd.iota(eoff_i[:], pattern=[[1, 1]], base=-1, channel_multiplier=C)
    eoff = sb.tile([8, 1], F32, tag="eoff")
    nc.vector.tensor_copy(eoff[:], eoff_i[:])
    nc.vector.tensor_scalar(slotA, rank8, eoff[:], None, op0=OP.add)
    nc.vector.scalar_tensor_tensor(s0A, g8, 0.5, slotA,
                                   op0=OP.is_ge, op1=OP.mult)
    nc.vector.tensor_tensor(s1A, mask8, slotA, op=OP.mult)
    nc.vector.tensor_tensor(s1A, s1A, s0A, op=OP.subtract)
    nc.vector.scalar_tensor_tensor(s0G, g8, 0.5, g8,
                                   op0=OP.is_ge, op1=OP.mult)

    # my-token window [tok0, tok0+196) via dynamic rhs slice
    treg = nc.tensor.alloc_register("tok0reg")
    nc.tensor.reg_load(treg, tok0_sb[0:1, 0:1])
    toff = nc.tensor.snap(treg, donate=True, min_val=0,
                          max_val=N * (NCORES - 1))

    c0my16 = sb.tile([1, N], I16, tag="c0my16")
    c1my16 = sb.tile([1, N], I16, tag="c1my16")
    g0my = sb.tile([1, N], F32, tag="g0my")
    with tc.tile_pool(name="psc", bufs=3, space="PSUM") as psc:
        for srcv, base, dst in ((s0A, 0, c0my16), (s1A, 0, c1my16),
                                (s0G, 0, g0my)):
            p = psc.tile([1, N], F32, tag="pscc")
            nc.tensor.matmul(p[:], ones_col[base:base + 8, 0:1],
                             srcv[:, bass.ds(toff, N)],
                             start=True, stop=True)
            nc.any.tensor_copy(dst[:], p[:])

    # combine idx: [c0 | pad | c1 | pad] -> dram[512] -> wrapped16 -> 128p
    c01 = sb.tile([1, 512], I16, tag="c01")
    nc.vector.memset(c01[:], 0)
    nc.vector.tensor_copy(c01[:, 0:N], c0my16[:])
    nc.vector.tensor_copy(c01[:, 256:256 + N], c1my16[:])
    cidx_d = dram.tile([1, 512], I16)
    nc.sync.dma_start(cidx_d[:], c01[:])
    cidx16 = sb.tile([16, 32], I16, tag="cidx16")
    nc.sync.dma_start(cidx16[:], cidx_d[:].rearrange("1 (f p) -> p f", p=16))
    cidx128 = sb.tile([128, 32], I16, tag="cidx128")
    _repl16(nc, cidx128, cidx16)

    # g0 window -> wrapped [128, 2]
    g0_d = dram.tile([1, N], F32)
    nc.scalar.dma_start(g0_d[:], g0my[:])
    g0w2 = sb.tile([128, 2], F32, tag="g0w2")
    nc.vector.memset(g0w2[:], 0.0)
    nc.sync.dma_start(g0w2[:, 0:1], g0_d[0:1, 0:128].rearrange("1 p -> p 1"))
    nc.sync.dma_start(g0w2[0:N - 128, 1:2],
                      g0_d[0:1, 128:N].rearrange("1 p -> p 1"))

    # dispatch idx: my expert row of mask8 -> compacted token list
    with tc.tile_pool(name="psm", bufs=2, space="PSUM") as psm:
        for c0, cw in ((0, 512), (512, 512), (1024, 512), (1536, 128)):
            p = psm.tile([1, cw], F32, tag="psmm")
            nc.tensor.matmul(p[:], eoh_sb[0:8, 0:1], mask8[:, c0:c0 + cw],
                             start=True, stop=True)
            nc.any.tensor_copy(mrow[0:1, c0:c0 + cw], p[:])
    mrow_d = dram.tile([1, TWPAD], F32)
    nc.scalar.dma_start(mrow_d[:], mrow)
    m16 = sb.tile([16, T // 16], F32, tag="m16")
    nc.scalar.dma_start(m16[:],
                        mrow_d[0:1, 0:T].rearrange("1 (f p) -> p f", p=16))
    iot = sb.tile([16, T // 16], I32, tag="iot")
    nc.gpsimd.iota(iot[:], pattern=[[16, T // 16]], base=0,
                   channel_multiplier=1)
    iotf = sb.tile([16, T // 16], F32, tag="iotf")
    nc.vector.tensor_copy(iotf[:], iot[:])
    cand = sb.tile([16, T // 16], F32, tag="cand")
    nc.vector.tensor_scalar_add(cand[:], iotf[:], 1.0)
    nc.vector.tensor_tensor(cand[:], cand[:], m16[:], op=OP.mult)
    nc.vector.tensor_scalar_add(cand[:], cand[:], -1.0)
    nfound = sb.tile([1, 1], U32, tag="nfound")
    comp = sb.tile([16, C // 16], F32, tag="comp")
    nc.gpsimd.sparse_gather(comp[:], cand[:], num_found=nfound[:])
    nc.vector.tensor_scalar(comp[:], comp[:], float(T - 1), 0.0,
                            op0=OP.min, op1=OP.max)
    didx16 = sb.tile([16, C // 16], I16, tag="didx16")
    nc.vector.tensor_copy(didx16[:], comp[:])
    didx128 = sb.tile([128, C // 16], I16, tag="didx128")
    _repl16(nc, didx128, didx16)

    if "rank8" in dbg:
        nc.sync.dma_start(dbg["rank8"][:], rank8)
    p_rt.release()

    # x1 token-major + b2 (fills the PE gap during routing/collectives)
    with tc.tile_pool(name="psr", bufs=2, space="PSUM") as psr:
        for i, sz in enumerate(NCH):
            pt = psr.tile([128, D], F32, tag="psrt")
            for ft in range(KT):
                nc.tensor.transpose(pt[:sz, 128 * ft:128 * (ft + 1)],
                                    x1T[:, ft, 128 * i:128 * i + sz],
                                    ident[:])
            nc.vector.tensor_tensor(x1tok[:sz, i, :], pt[:sz, :],
                                    b2_b[:sz, :], op=OP.add)

    # =====================================================================
    # Phase D: dispatch gather, expert FFN, AllGather #2
    # =====================================================================
    y_d = dram.tile([C, D], BF16)
    p_hT = tc.alloc_tile_pool(name="p_hT", bufs=1)
    with tc.tile_pool(name="p_w1", bufs=1) as pw1:
        w1_sb = pw1.tile([128, KT, HID], F32R, tag="w1")
        for k in range(KT):
            nc.sync.dma_start(w1_sb[:, k, :],
                              ins["w1"][128 * k:128 * (k + 1), :]
                              .bitcast(F32R))
        XgT = pw1.tile([128, KT, C], F32R, tag="XgT")
        with tc.tile_pool(name="p_xg", bufs=1) as pxg:
            Xg = pxg.tile([128, CM, D], BF16, tag="Xg")
            nc.gpsimd.dma_gather(Xg[:], ag_out[:], didx128[:],
                                 num_idxs=C, num_idxs_reg=C, elem_size=D)
            with tc.tile_pool(name="psx", bufs=3, space="PSUM") as psx:
                for ft in range(KT):
                    pt = psx.tile([128, C], BF16, tag="psxt")
                    for ct in range(CM):
                        nc.tensor.transpose(
                            pt[:, 128 * ct:128 * (ct + 1)],
                            Xg[:, ct, 128 * ft:128 * (ft + 1)], ident_bf[:])
                    nc.any.tensor_copy(XgT[:, ft, :], pt[:])

        hT = p_hT.tile([128, HM, C], F32R, tag="hT")
        with tc.tile_pool(name="ps1", bufs=3, space="PSUM") as ps1:
            for m in range(HM):
                ph = ps1.tile([128, C], F32, tag="ps1t")
                for k in range(KT):
                    nc.tensor.matmul(ph[:],
                                     w1_sb[:, k, 128 * m:128 * (m + 1)],
                                     XgT[:, k, :], start=(k == 0),
                                     stop=(k == KT - 1))
                nc.scalar.activation(hT[:, m, :], ph[:], AF.Gelu,
                                     bias=cvec["b1"][:, m:m + 1])

    yall = dram.tile([NCORES * C, D], BF16, addr_space="Shared")
    with (
        tc.tile_pool(name="p_y", bufs=1) as py,
        tc.tile_pool(name="ps4", bufs=1, space="PSUM") as ps4,
        tc.tile_pool(name="w2p", bufs=3) as w2p,
    ):
        y_sb = py.tile([128, CM, D], BF16, tag="y_sb")
        yps = {}
        for mc in range(CM):
            for nb in range(2):
                ypst = ps4.tile([128, 384], F32, tag=f"y{mc}{nb}")
                yps[(mc, nb)] = ypst
        for k in range(HM):
            w2t = w2p.tile([128, D], F32R, tag="w2t")
            nc.sync.dma_start(w2t[:],
                              ins["w2"][128 * k:128 * (k + 1), :]
                              .bitcast(F32R))
            for mc in range(CM):
                for nb in range(2):
                    nc.tensor.matmul(
                        yps[(mc, nb)][:],
                        hT[:, k, 128 * mc:128 * (mc + 1)],
                        w2t[:, 384 * nb:384 * (nb + 1)],
                        start=(k == 0), stop=(k == HM - 1))
        for mc in range(CM):
            for nb in range(2):
                nc.any.tensor_copy(y_sb[:, mc, 384 * nb:384 * (nb + 1)],
                                   yps[(mc, nb)][:])
        nc.sync.dma_start(y_d[:].rearrange("(i p) d -> p i d", p=128),
                          y_sb[:])
        if "ysb" in dbg:
            nc.sync.dma_start(
                dbg["ysb"][:].rearrange("(i p) d -> p i d", p=128), y_sb[:])
    nc.gpsimd.collective_compute(
        "AllGather", OP.bypass, ins=[y_d.opt()], outs=[yall.opt()],
        replica_groups=[list(range(NCORES))])

    p_hT.release()

    # =====================================================================
    # Phase E: combine gather + weighted sum + output
    # =====================================================================
    with tc.tile_pool(name="p_e", bufs=1) as pe:
        g_tok = pe.tile([128, 4, D], BF16, tag="g_tok")
        nc.gpsimd.dma_gather(g_tok[:], yall[:], cidx128[:],
                             num_idxs=512, num_idxs_reg=512, elem_size=D)
        for i, sz in enumerate(NCH):
            dtile = sbw.tile([128, D], F32, tag="cmb_d")
            nc.vector.tensor_tensor(dtile[:sz, :], g_tok[:sz, i, :],
                                    g_tok[:sz, i + 2, :], op=OP.subtract)
            t2 = sbw.tile([128, D], F32, tag="cmb_t2")
            nc.vector.scalar_tensor_tensor(t2[:sz, :], dtile[:sz, :],
                                           g0w2[:sz, i:i + 1],
                                           g_tok[:sz, i + 2, :],
                                           op0=OP.mult, op1=OP.add)
            ocmb = sbw.tile([128, D], F32, tag="cmb_o")
            nc.vector.tensor_tensor(ocmb[:sz, :], t2[:sz, :],
                                    x1tok[:sz, i, :], op=OP.add)
            nc.sync.dma_start(out[128 * i:128 * i + sz, :], ocmb[:sz, :])

    # remaining debug taps
    if "gates" in dbg:
        for i in range(TW):
            nc.sync.dma_start(dbg["gates"][0:128, 8 * i:8 * (i + 1)],
                              gates[:, i, :])
    if "cidx" in dbg:
        nc.sync.dma_start(dbg["cidx"][:], cidx128[0:16, :])
    if "didx" in dbg:
        nc.sync.dma_start(dbg["didx"][:], didx128[0:16, :])
    if "g0w2" in dbg:
        nc.sync.dma_start(dbg["g0w2"][:], g0w2[:])
    if "x1T" in dbg:
        for k in range(KT):
            nc.sync.dma_start(dbg["x1T"][128 * k:128 * (k + 1), :],
                              x1T[:, k, :])

    sbw.release()
    sb.release()
    dram.release()


# ---------------------------------------------------------------------------
# host side
# ---------------------------------------------------------------------------

DBG_SPECS = {
    "xnT": (D, N), "gates": (128, TW * 8), "rank8": (8, TWPAD),
    "cidx": (16, 32), "didx": (16, C // 16), "g0w2": (128, 2),
    "ysb": (C, D), "x1T": (D, N),
}

_PROGRAM_CACHE = {}


def get_program(debug=False):
    key = bool(debug)
    if key in _PROGRAM_CACHE:
        return _PROGRAM_CACHE[key]
    nc = bacc.Bacc("TRN2", target_bir_lowering=False, debug=False,
                   enable_asserts=True, num_devices=NCORES)
    ins = {}

    def din(name, shape, dtype=F32):
        ins[name] = nc.dram_tensor(name, list(shape), dtype,
                                   kind="ExternalInput").ap()

    din("xT", (D, N))
    din("qkv_w", (D, 3 * D))
    din("qkv_b", (128, 18))
    din("proj_w", (D, D))
    din("proj_b", (128, KT))
    din("ln1_g", (128, KT))
    din("ln1_b", (128, KT))
    din("ln2_g", (128, KT))
    din("ln2_b", (128, KT))
    din("gate_w", (D, E))
    din("w1", (D, HID))
    din("b1", (128, HM))
    din("w2", (HID, D))
    din("b2", (1, D))
    din("tok0", (1, 1), I32)
    din("eoh", (128, 1))

    outs = {"out": nc.dram_tensor("out", [N, D], F32,
                                  kind="ExternalOutput").ap()}
    if debug:
        outs["dbg"] = {
            nm: nc.dram_tensor(f"dbg_{nm}", list(shp),
                               I16 if nm in ("cidx", "didx")
                               else (BF16 if nm == "ysb" else F32),
                               kind="ExternalOutput").ap()
            for nm, shp in DBG_SPECS.items()
        }

    with tile.TileContext(nc) as tc:
        build_block(tc, outs, ins)
    nc.compile()
    _PROGRAM_CACHE[key] = nc
    return nc


def round_f32r(x):
    """Round fp32 to fp32r (8e11m) with round-to-nearest-even."""
    b = np.ascontiguousarray(x, np.float32).view(np.uint32).copy()
    lsb = (b >> np.uint32(12)) & np.uint32(1)
    r = b + np.uint32(0x7FF) + lsb
    return (r & np.uint32(0xFFFFF000)).view(np.float32)


def prep_in_maps(inputs):
    f = {k: np.ascontiguousarray(np.asarray(v), dtype=np.float32)
         for k, v in inputs.items()}
    share = {
        "qkv_w": f["qkv_w"],
        "qkv_b": f["qkv_b"].reshape(18, 128).T.copy(),
        "proj_w": f["proj_w"],
        "proj_b": f["proj_b"].reshape(KT, 128).T.copy(),
        "ln1_g": f["ln1_g"].reshape(KT, 128).T.copy(),
        "ln1_b": f["ln1_b"].reshape(KT, 128).T.copy(),
        "ln2_g": f["ln2_g"].reshape(KT, 128).T.copy(),
        "ln2_b": f["ln2_b"].reshape(KT, 128).T.copy(),
        "gate_w": f["gate_w"],
    }
    in_maps = []
    for o in range(NCORES):
        m = dict(share)
        m["xT"] = f["x"][o].T.copy()
        m["w1"] = round_f32r(f["w1"][o])
        m["b1"] = f["b1"][o].reshape(HM, 128).T.copy()
        m["w2"] = round_f32r(f["w2"][o])
        m["b2"] = f["b2"][o].reshape(1, D).copy()
        m["tok0"] = np.array([[o * N]], np.int32)
        eoh = np.zeros((8, 1), np.float32)
        eoh[o, 0] = 1.0
        m["eoh"] = np.tile(eoh, (16, 1))
        in_maps.append(m)
    return in_maps


def kernel(**inputs):
    nc = get_program(debug=False)
    in_maps = prep_in_maps(inputs)
    res = bass_utils.run_bass_kernel_spmd(
        nc, in_maps, core_ids=list(range(NCORES)), trace=False)
    out = np.stack([r["out"] for r in res.results], axis=0)
    return out.astype(np.float32)


# revision 21
# speedup vs baseline: 1.4180x; 1.0573x over previous
"""MoE transformer block (attention + top-2 routed 8-expert FFN) on 8 TRN2
NeuronCores.

Sharding: data-parallel attention (1 image of 196 tokens per core) +
expert-parallel MoE (1 expert per core). Token dispatch/combine via
AllGather + dma_gather, with slot ranks computed on-device by a cumsum
scan over the routing mask.

Self-contained: hardcodes all shapes; imports only concourse (already on
PYTHONPATH in the runtime image).
"""
import os
import sys
for _p in ("/opt/trn_rl_repo", "/root/.axon_site/_ro/trn_rl_repo"):
    if _p not in sys.path:
        sys.path.append(_p)

import numpy as np
import concourse.bass as bass
import concourse.bacc as bacc
import concourse.mybir as mybir
import concourse.tile as tile
from concourse import masks
from concourse import bass_utils

F32 = mybir.dt.float32
F32R = mybir.dt.float32r
BF16 = mybir.dt.bfloat16
I16 = mybir.dt.int16
I32 = mybir.dt.int32
U32 = mybir.dt.uint32
AF = mybir.ActivationFunctionType
OP = mybir.AluOpType

NCORES = 8
B, N, D = 8, 196, 768
HEADS, HD = 12, 64
E, HID = 8, 3072
T = B * N                  # 1568
C = 512                    # expert capacity (max observed count is 424)
KT = D // 128              # 6 feature k-tiles
HM = HID // 128            # 24 hidden tiles
CM = C // 128              # 4 capacity tiles
LN_EPS = 1e-5
ROWB = 832                 # AG row: 768 xn + 8 logits + pad to 256B multiple
TW = 13                    # ceil(T/128) wrapped token tiles
TWPAD = TW * 128           # 1664
NCH = [128, N - 128]       # valid partitions per token chunk (128 + 68)


def _repl16(nc, dst, src16):
    """Replicate a [16, f] tile across all 128 partitions of dst."""
    nc.vector.memset(dst[:], 0)
    nc.vector.tensor_copy(dst[0:16, :], src16[:])
    nc.vector.stream_shuffle(dst[0:32, :], dst[0:32, :],
                             mask=[i % 16 for i in range(32)])
    nc.vector.tensor_copy(dst[32:64, :], dst[0:32, :])
    nc.vector.tensor_copy(dst[64:128, :], dst[0:64, :])


def build_block(tc, outs, ins):
    nc = tc.nc
    out = outs["out"]          # [196, 768] per-core output
    dbg = outs.get("dbg", {})  # optional {name: AP} debug outputs

    sb = tc.alloc_tile_pool(name="sb", bufs=1)      # small persistents
    sbw = tc.alloc_tile_pool(name="sbw", bufs=3)    # loop workspace
    dram = tc.alloc_tile_pool(name="dram", bufs=1, space="DRAM")

    ident = sb.tile([128, 128], F32)
    masks.make_identity(nc, ident[:])
    ident_bf = sb.tile([128, 128], BF16)
    nc.vector.tensor_copy(ident_bf[:], ident[:])
    ones_col = sb.tile([128, 1], F32)
    nc.vector.memset(ones_col[:], 1.0)
    ones8 = sb.tile([8, 1], F32)
    nc.vector.memset(ones8[:], 1.0)
    eps_sb = sb.tile([1, 1], F32)
    nc.vector.memset(eps_sb[:], LN_EPS)

    cvec = {}
    for nm, w in (("qkv_b", 18), ("proj_b", KT), ("ln1_g", KT), ("ln1_b", KT),
                  ("ln2_g", KT), ("ln2_b", KT), ("b1", HM)):
        t = sb.tile([128, w], F32, tag=nm)
        nc.sync.dma_start(t[:], ins[nm][:])
        cvec[nm] = t
    b2_row = sb.tile([1, D], F32, tag="b2r")
    nc.sync.dma_start(b2_row[:], ins["b2"][:])
    b2_b = sb.tile([128, D], F32, tag="b2b")
    nc.gpsimd.partition_broadcast(b2_b[:], b2_row[:])

    tok0_sb = sb.tile([1, 1], I32, tag="tok0")
    nc.sync.dma_start(tok0_sb[:], ins["tok0"][:])
    eoh_sb = sb.tile([128, 1], F32, tag="eoh")
    nc.sync.dma_start(eoh_sb[:], ins["eoh"][:])

    # medium-lived persistents
    x1T = sb.tile([128, KT, N], F32, tag="x1T")
    x1tok = sb.tile([128, 2, D], F32, tag="x1tok")

    # ---- feature-major LayerNorm via matmul-ones stats -------------------
    def layer_norm_fm(src, g_t, b_t, dst, psln, wk):
        s1 = psln.tile([1, N], F32, tag="lnp1")
        for k in range(KT):
            nc.tensor.matmul(s1[:], ones_col[:], src[:, k, :],
                             start=(k == 0), stop=(k == KT - 1))
        s2 = psln.tile([1, N], F32, tag="lnp2")
        for k in range(KT):
            sq = wk.tile([128, N], F32, tag="ln_sq")
            nc.scalar.activation(sq[:], src[:, k, :], AF.Square)
            nc.tensor.matmul(s2[:], ones_col[:], sq[:],
                             start=(k == 0), stop=(k == KT - 1))
        mu = wk.tile([1, N], F32, tag="ln_mu")
        nc.vector.tensor_scalar_mul(mu[:], s1[:], 1.0 / D)
        nmu2 = wk.tile([1, N], F32, tag="ln_nmu2")
        nc.vector.scalar_tensor_tensor(nmu2[:], mu[:], -1.0, mu[:],
                                       op0=OP.mult, op1=OP.mult)
        var = wk.tile([1, N], F32, tag="ln_var")
        nc.vector.scalar_tensor_tensor(var[:], s2[:], 1.0 / D, nmu2[:],
                                       op0=OP.mult, op1=OP.add)
        sd = wk.tile([1, N], F32, tag="ln_sd")
        nc.scalar.activation(sd[:], var[:], AF.Sqrt, bias=eps_sb[:])
        rstd = wk.tile([1, N], F32, tag="ln_rstd")
        nc.vector.reciprocal(rstd[:], sd[:])
        mu_b = wk.tile([128, N], F32, tag="ln_mub")
        nc.gpsimd.partition_broadcast(mu_b[:], mu[:])
        rstd_b = wk.tile([128, N], F32, tag="ln_rstdb")
        nc.gpsimd.partition_broadcast(rstd_b[:], rstd[:])
        for k in range(KT):
            tmp = wk.tile([128, N], F32, tag="ln_tmp")
            nc.vector.tensor_tensor(tmp[:], src[:, k, :], mu_b[:],
                                    op=OP.subtract)
            nc.vector.tensor_tensor(tmp[:], tmp[:], rstd_b[:], op=OP.mult)
            nc.any.tensor_scalar(dst[:, k, :], tmp[:], g_t[:, k:k + 1],
                                 b_t[:, k:k + 1], op0=OP.mult, op1=OP.add)

    # =====================================================================
    # Phase A: attention (fp32, feature-major)
    # =====================================================================
    with (
        tc.tile_pool(name="p_attn", bufs=1) as pa,
        tc.tile_pool(name="p_qkvw", bufs=1) as pqw,
    ):
        xT_sb = pa.tile([128, KT, N], F32, tag="xT")
        projw_sb = pa.tile([128, KT, D], F32, tag="projw")
        qkvw_sb = pqw.tile([128, KT, 2304], F32, tag="qkvw")
        for k in range(KT):
            nc.sync.dma_start(xT_sb[:, k, :],
                              ins["xT"][128 * k:128 * (k + 1), :])
        for k in range(KT):
            nc.sync.dma_start(qkvw_sb[:, k, :],
                              ins["qkv_w"][128 * k:128 * (k + 1), :])
        for k in range(KT):
            nc.sync.dma_start(projw_sb[:, k, :],
                              ins["proj_w"][128 * k:128 * (k + 1), :])

        xn1 = pa.tile([128, KT, N], F32, tag="xn1")
        with (
            tc.tile_pool(name="psln", bufs=1, space="PSUM") as psln,
            tc.tile_pool(name="wkln1", bufs=2) as wkln1,
        ):
            layer_norm_fm(xT_sb, cvec["ln1_g"], cvec["ln1_b"], xn1, psln,
                          wkln1)

        qkvT = pa.tile([128, 18, N], F32, tag="qkvT")
        with tc.tile_pool(name="psqkv", bufs=3, space="PSUM") as psqkv:
            for m in range(18):
                ps = psqkv.tile([128, N], F32, tag="qkvps")
                for k in range(KT):
                    nc.tensor.matmul(
                        ps[:], qkvw_sb[:, k, 128 * m:128 * (m + 1)],
                        xn1[:, k, :], start=(k == 0), stop=(k == KT - 1))
                nc.any.tensor_scalar(qkvT[:, m, :], ps[:],
                                     cvec["qkv_b"][:, m:m + 1], None,
                                     op0=OP.add)

        def head_slice(base, h):
            return qkvT[64 * (h % 2):64 * (h % 2) + 64, base + h // 2, :]

        attn_out = pa.tile([128, 2, D], F32, tag="attn_out")  # token-major
        with (
            tc.tile_pool(name="pss", bufs=2, space="PSUM") as pss,
            tc.tile_pool(name="pst", bufs=3, space="PSUM") as pst,
            tc.tile_pool(name="pso", bufs=2, space="PSUM") as pso,
            tc.tile_pool(name="wka", bufs=3) as wka,
        ):
            for h in range(HEADS):
                qT, kT, vT = (head_slice(b, h) for b in (0, 6, 12))
                b0 = HD * (h % 2)
                v_tok = wka.tile([128, 2, HD], F32, tag="v_tok")
                for i, sz in enumerate(NCH):
                    pv = pst.tile([128, 128], F32, tag="ptt")
                    nc.tensor.transpose(pv[:sz, :HD],
                                        vT[:, 128 * i:128 * i + sz],
                                        ident[b0:b0 + HD, b0:b0 + HD])
                    nc.any.tensor_copy(v_tok[:sz, i, :], pv[:sz, :HD])
                attnT = wka.tile([128, 2, N], F32, tag="attnT")
                rcp = wka.tile([128, 2], F32, tag="rcp")
                for i, sz in enumerate(NCH):
                    ps = pss.tile([128, N], F32, tag="sps")
                    nc.tensor.matmul(ps[:sz, :], qT[:, 128 * i:128 * i + sz],
                                     kT[:], start=True, stop=True)
                    m0 = wka.tile([128, 1], F32, tag="sm_m0")
                    nc.vector.tensor_reduce(m0[:sz], ps[:sz, :],
                                            axis=mybir.AxisListType.X,
                                            op=OP.max)
                    m0n = wka.tile([128, 1], F32, tag="sm_m0n")
                    nc.vector.tensor_scalar_mul(m0n[:sz], m0[:sz], -0.125)
                    ex = wka.tile([128, N], F32, tag="sm_ex")
                    ssum = wka.tile([128, 1], F32, tag="sm_sum")
                    nc.scalar.activation(ex[:sz, :], ps[:sz, :], AF.Exp,
                                         bias=m0n[:sz], scale=0.125,
                                         accum_out=ssum[:sz])
                    nc.vector.reciprocal(rcp[:sz, i:i + 1], ssum[:sz])
                    for j, szj in enumerate(NCH):
                        pt = pst.tile([128, 128], F32, tag="ptt")
                        nc.tensor.transpose(pt[:szj, :sz],
                                            ex[:sz, 128 * j:128 * j + szj],
                                            ident[0:sz, 0:sz])
                        nc.any.tensor_copy(
                            attnT[:szj, j, 128 * i:128 * i + sz],
                            pt[:szj, :sz])
                for i, sz in enumerate(NCH):
                    po = pso.tile([128, HD], F32, tag="pso")
                    for j, szj in enumerate(NCH):
                        nc.tensor.matmul(po[:sz, :],
                                         attnT[:szj, j, 128 * i:128 * i + sz],
                                         v_tok[:szj, j, :],
                                         start=(j == 0), stop=(j == 1))
                    nc.any.tensor_scalar(
                        attn_out[:sz, i, 64 * h:64 * (h + 1)], po[:sz, :],
                        rcp[:sz, i:i + 1], None, op0=OP.mult)

        aoT = pa.tile([128, KT, N], F32, tag="aoT")
        with (
            tc.tile_pool(name="psat", bufs=3, space="PSUM") as psat,
            tc.tile_pool(name="psp", bufs=3, space="PSUM") as psp,
        ):
            for ft in range(KT):
                pt = psat.tile([128, N], F32, tag="psatt")
                for i, sz in enumerate(NCH):
                    nc.tensor.transpose(
                        pt[:, 128 * i:128 * i + sz],
                        attn_out[:sz, i, 128 * ft:128 * (ft + 1)],
                        ident[0:sz, 0:sz])
                nc.any.tensor_copy(aoT[:, ft, :], pt[:])
            for ft in range(KT):
                ps = psp.tile([128, N], F32, tag="pspp")
                for k in range(KT):
                    nc.tensor.matmul(ps[:],
                                     projw_sb[:, k, 128 * ft:128 * (ft + 1)],
                                     aoT[:, k, :], start=(k == 0),
                                     stop=(k == KT - 1))
                nc.vector.scalar_tensor_tensor(
                    x1T[:, ft, :], ps[:], cvec["proj_b"][:, ft:ft + 1],
                    xT_sb[:, ft, :], op0=OP.add, op1=OP.add)

    # =====================================================================
    # Phase B: LN2, bounce assembly, AllGather #1
    # =====================================================================
    ag_in = dram.tile([N, D], BF16)
    ag_out = dram.tile([T, D], BF16, addr_space="Shared")
    agl_in = dram.tile([N, 8], F32)
    agl_out = dram.tile([T, 8], F32, addr_space="Shared")
    with tc.tile_pool(name="p_gate", bufs=1) as pg:
        gatew_sb = pg.tile([128, KT, 8], F32, tag="gatew")
        for k in range(KT):
            nc.sync.dma_start(gatew_sb[:, k, :],
                              ins["gate_w"][128 * k:128 * (k + 1), :])
        xnT = pg.tile([128, KT, N], F32, tag="xnT")
        with (
            tc.tile_pool(name="psln2", bufs=1, space="PSUM") as psln2,
            tc.tile_pool(name="wkln2", bufs=2) as wkln2,
        ):
            layer_norm_fm(x1T, cvec["ln2_g"], cvec["ln2_b"], xnT, psln2,
                          wkln2)

        bounce = pg.tile([128, 2, D], BF16, tag="bounce")
        blog = pg.tile([128, 2, 8], F32, tag="blog")
        with tc.tile_pool(name="psb", bufs=2, space="PSUM") as psb:
            for i, sz in enumerate(NCH):
                pl = psb.tile([128, 8], F32, tag="psbl")
                for k in range(KT):
                    nc.tensor.matmul(pl[:sz, :],
                                     xnT[:, k, 128 * i:128 * i + sz],
                                     gatew_sb[:, k, :], start=(k == 0),
                                     stop=(k == KT - 1))
                nc.any.tensor_copy(blog[:sz, i, :], pl[:sz, :])
            for i, sz in enumerate(NCH):
                nc.sync.dma_start(agl_in[128 * i:128 * i + sz, :],
                                  blog[:sz, i, :])
            for i, sz in enumerate(NCH):
                pt = psb.tile([128, D], F32, tag="psbt")
                for ft in range(KT):
                    nc.tensor.transpose(pt[:sz, 128 * ft:128 * (ft + 1)],
                                        xnT[:, ft, 128 * i:128 * i + sz],
                                        ident[:])
                nc.any.tensor_copy(bounce[:sz, i, 0:D], pt[:sz, :])
        for i, sz in enumerate(NCH):
            nc.sync.dma_start(ag_in[128 * i:128 * i + sz, :],
                              bounce[:sz, i, :])
        if "xnT" in dbg:
            for k in range(KT):
                nc.sync.dma_start(dbg["xnT"][128 * k:128 * (k + 1), :],
                                  xnT[:, k, :])
    nc.gpsimd.collective_compute(
        "AllGather", OP.bypass, ins=[agl_in.opt()], outs=[agl_out.opt()],
        replica_groups=[list(range(NCORES))])
    nc.gpsimd.collective_compute(
        "AllGather", OP.bypass, ins=[ag_in.opt()], outs=[ag_out.opt()],
        replica_groups=[list(range(NCORES))])

    # =====================================================================
    # Phase C: routing (replicated on every core)
    # =====================================================================
    lg = sb.tile([128, TW, 8], F32, tag="lg")
    nc.vector.memset(lg[:], 0.0)
    nc.sync.dma_start(
        lg[:, 0:12, :],
        agl_out[0:1536, :].rearrange("(i p) c -> p i c", p=128))
    nc.sync.dma_start(lg[0:32, 12, :], agl_out[1536:T, :])

    gates = sb.tile([128, TW, 8], F32, tag="gates")
    g0wr = sb.tile([128, TW], F32, tag="g0wr")
    m0 = sb.tile([128, TW], F32, tag="rt_m0")
    nc.vector.tensor_reduce(m0[:].rearrange("p (i o) -> p i o", o=1), lg[:],
                            axis=mybir.AxisListType.X, op=OP.max)
    m0v = m0[:].rearrange("p (i o) -> p i o", o=1).to_broadcast([128, TW, 8])
    mk0 = sb.tile([128, TW, 8], F32, tag="rt_mk0")
    nc.vector.tensor_tensor(mk0[:], lg[:], m0v, op=OP.is_ge)
    msk = sb.tile([128, TW, 8], F32, tag="rt_msk")
    nc.vector.scalar_tensor_tensor(msk[:], mk0[:], -1e30, lg[:],
                                   op0=OP.mult, op1=OP.add)
    m1 = sb.tile([128, TW], F32, tag="rt_m1")
    nc.vector.tensor_reduce(m1[:].rearrange("p (i o) -> p i o", o=1), msk[:],
                            axis=mybir.AxisListType.X, op=OP.max)
    m1v = m1[:].rearrange("p (i o) -> p i o", o=1).to_broadcast([128, TW, 8])
    exd = sb.tile([128, TW, 8], F32, tag="rt_exd")
    nc.vector.tensor_tensor(exd[:], lg[:], m0v, op=OP.subtract)
    ex = sb.tile([128, TW, 8], F32, tag="rt_ex")
    nc.scalar.activation(ex[:], exd[:], AF.Exp)
    e1d = sb.tile([128, TW], F32, tag="rt_e1d")
    nc.vector.tensor_tensor(e1d[:], m1[:], m0[:], op=OP.subtract)
    e1 = sb.tile([128, TW], F32, tag="rt_e1")
    nc.scalar.activation(e1[:], e1d[:], AF.Exp)
    den = sb.tile([128, TW], F32, tag="rt_den")
    nc.vector.tensor_scalar_add(den[:], e1[:], 1.0)
    nc.vector.reciprocal(g0wr[:], den[:])
    g0v = g0wr[:].rearrange("p (i o) -> p i o", o=1).to_broadcast([128, TW, 8])
    mt = sb.tile([128, TW, 8], F32, tag="rt_mt")
    nc.vector.tensor_tensor(mt[:], lg[:], m1v, op=OP.is_ge)
    gu = sb.tile([128, TW, 8], F32, tag="rt_gu")
    nc.vector.tensor_tensor(gu[:], ex[:], mt[:], op=OP.mult)
    nc.vector.tensor_tensor(gates[:], gu[:], g0v, op=OP.mult)

    p_rt = tc.alloc_tile_pool(name="p_rt", bufs=1)
    g8_t = p_rt.tile([8, TWPAD], F32, tag="g8")
    mask8_t = p_rt.tile([8, TWPAD], F32, tag="mask8")
    rank8_t = p_rt.tile([8, TWPAD], F32, tag="rank8")
    slotA_t = p_rt.tile([8, TWPAD], F32, tag="slotA")
    s0A_t = p_rt.tile([8, TWPAD], F32, tag="s0A")
    s1A_t = p_rt.tile([8, TWPAD], F32, tag="s1A")
    s0G_t = p_rt.tile([8, TWPAD], F32, tag="s0G")
    mrow_t = p_rt.tile([1, TWPAD], F32, tag="mrow")
    g8 = g8_t[:]
    mask8 = mask8_t[:]
    rank8 = rank8_t[:]
    slotA = slotA_t[:]
    s0A = s0A_t[:]
    s1A = s1A_t[:]
    s0G = s0G_t[:]
    mrow = mrow_t[:]
    with tc.tile_pool(name="psg", bufs=3, space="PSUM") as psg:
        for i in range(TW):
            pt = psg.tile([8, 128], F32, tag="psgt")
            nc.tensor.transpose(pt[:], gates[:, i, :], ident[:])
            nc.any.tensor_copy(g8[:, 128 * i:128 * (i + 1)], pt[:])

    nc.vector.tensor_scalar(mask8, g8, 0.0, None, op0=OP.is_gt)
    nc.vector.tensor_tensor_scan(rank8, mask8, mask8, initial=0.0,
                                 op0=OP.add, op1=OP.bypass)
    eoff_i = sb.tile([8, 1], I32, tag="eoff_i")
    nc.gpsimd.iota(eoff_i[:], pattern=[[1, 1]], base=-1, channel_multiplier=C)
    eoff = sb.tile([8, 1], F32, tag="eoff")
    nc.vector.tensor_copy(eoff[:], eoff_i[:])
    nc.vector.tensor_scalar(slotA, rank8, eoff[:], None, op0=OP.add)
    nc.vector.scalar_tensor_tensor(s0A, g8, 0.5, slotA,
                                   op0=OP.is_ge, op1=OP.mult)
    nc.vector.tensor_tensor(s1A, mask8, slotA, op=OP.mult)
    nc.vector.tensor_tensor(s1A, s1A, s0A, op=OP.subtract)
    nc.vector.scalar_tensor_tensor(s0G, g8, 0.5, g8,
                                   op0=OP.is_ge, op1=OP.mult)

    # my-token window [tok0, tok0+196) via dynamic rhs slice
    treg = nc.tensor.alloc_register("tok0reg")
    nc.tensor.reg_load(treg, tok0_sb[0:1, 0:1])
    toff = nc.tensor.snap(treg, donate=True, min_val=0,
                          max_val=N * (NCORES - 1))

    c0my16 = sb.tile([1, N], I16, tag="c0my16")
    c1my16 = sb.tile([1, N], I16, tag="c1my16")
    g0my = sb.tile([1, N], F32, tag="g0my")
    with tc.tile_pool(name="psc", bufs=3, space="PSUM") as psc:
        for srcv, base, dst in ((s0A, 0, c0my16), (s1A, 0, c1my16),
                                (s0G, 0, g0my)):
            p = psc.tile([1, N], F32, tag="pscc")
            nc.tensor.matmul(p[:], ones_col[base:base + 8, 0:1],
                             srcv[:, bass.ds(toff, N)],
                             start=True, stop=True)
            nc.any.tensor_copy(dst[:], p[:])

    # combine idx: [c0 | pad | c1 | pad] -> dram[512] -> wrapped16 -> 128p
    c01 = sb.tile([1, 512], I16, tag="c01")
    nc.vector.memset(c01[:], 0)
    nc.vector.tensor_copy(c01[:, 0:N], c0my16[:])
    nc.vector.tensor_copy(c01[:, 256:256 + N], c1my16[:])
    cidx_d = dram.tile([1, 512], I16)
    nc.sync.dma_start(cidx_d[:], c01[:])
    cidx16 = sb.tile([16, 32], I16, tag="cidx16")
    nc.sync.dma_start(cidx16[:], cidx_d[:].rearrange("1 (f p) -> p f", p=16))
    cidx128 = sb.tile([128, 32], I16, tag="cidx128")
    _repl16(nc, cidx128, cidx16)

    # g0 window -> wrapped [128, 2]
    g0_d = dram.tile([1, N], F32)
    nc.scalar.dma_start(g0_d[:], g0my[:])
    g0w2 = sb.tile([128, 2], F32, tag="g0w2")
    nc.vector.memset(g0w2[:], 0.0)
    nc.sync.dma_start(g0w2[:, 0:1], g0_d[0:1, 0:128].rearrange("1 p -> p 1"))
    nc.sync.dma_start(g0w2[0:N - 128, 1:2],
                      g0_d[0:1, 128:N].rearrange("1 p -> p 1"))

    # dispatch idx: my expert row of mask8 -> compacted token list
    with tc.tile_pool(name="psm", bufs=2, space="PSUM") as psm:
        for c0, cw in ((0, 512), (512, 512), (1024, 512), (1536, 128)):
            p = psm.tile([1, cw], F32, tag="psmm")
            nc.tensor.matmul(p[:], eoh_sb[0:8, 0:1], mask8[:, c0:c0 + cw],
                             start=True, stop=True)
            nc.any.tensor_copy(mrow[0:1, c0:c0 + cw], p[:])
    mrow_d = dram.tile([1, TWPAD], F32)
    nc.scalar.dma_start(mrow_d[:], mrow)
    m16 = sb.tile([16, T // 16], F32, tag="m16")
    nc.scalar.dma_start(m16[:],
                        mrow_d[0:1, 0:T].rearrange("1 (f p) -> p f", p=16))
    iot = sb.tile([16, T // 16], I32, tag="iot")
    nc.gpsimd.iota(iot[:], pattern=[[16, T // 16]], base=0,
                   channel_multiplier=1)
    iotf = sb.tile([16, T // 16], F32, tag="iotf")
    nc.vector.tensor_copy(iotf[:], iot[:])
    cand = sb.tile([16, T // 16], F32, tag="cand")
    nc.vector.tensor_scalar_add(cand[:], iotf[:], 1.0)
    nc.vector.tensor_tensor(cand[:], cand[:], m16[:], op=OP.mult)
    nc.vector.tensor_scalar_add(cand[:], cand[:], -1.0)
    nfound = sb.tile([1, 1], U32, tag="nfound")
    comp = sb.tile([16, C // 16], F32, tag="comp")
    nc.gpsimd.sparse_gather(comp[:], cand[:], num_found=nfound[:])
    nc.vector.tensor_scalar(comp[:], comp[:], float(T - 1), 0.0,
                            op0=OP.min, op1=OP.max)
    didx16 = sb.tile([16, C // 16], I16, tag="didx16")
    nc.vector.tensor_copy(didx16[:], comp[:])
    didx128 = sb.tile([128, C // 16], I16, tag="didx128")
    _repl16(nc, didx128, didx16)

    if "rank8" in dbg:
        nc.sync.dma_start(dbg["rank8"][:], rank8)
    p_rt.release()

    # x1 token-major + b2 (fills the PE gap during routing/collectives)
    with tc.tile_pool(name="psr", bufs=2, space="PSUM") as psr:
        for i, sz in enumerate(NCH):
            pt = psr.tile([128, D], F32, tag="psrt")
            for ft in range(KT):
                nc.tensor.transpose(pt[:sz, 128 * ft:128 * (ft + 1)],
                                    x1T[:, ft, 128 * i:128 * i + sz],
                                    ident[:])
            nc.vector.tensor_tensor(x1tok[:sz, i, :], pt[:sz, :],
                                    b2_b[:sz, :], op=OP.add)

    # =====================================================================
    # Phase D: dispatch gather, expert FFN, AllGather #2
    # =====================================================================
    y_d = dram.tile([C, D], BF16)
    p_hT = tc.alloc_tile_pool(name="p_hT", bufs=1)
    with tc.tile_pool(name="p_w1", bufs=1) as pw1:
        w1_sb = pw1.tile([128, KT, HID], BF16, tag="w1")
        for k in range(KT):
            nc.sync.dma_start(w1_sb[:, k, :],
                              ins["w1"][128 * k:128 * (k + 1), :])
        XgT = pw1.tile([128, KT, C], BF16, tag="XgT")
        with tc.tile_pool(name="p_xg", bufs=1) as pxg:
            Xg = pxg.tile([128, CM, D], BF16, tag="Xg")
            nc.gpsimd.dma_gather(Xg[:], ag_out[:], didx128[:],
                                 num_idxs=C, num_idxs_reg=C, elem_size=D)
            with tc.tile_pool(name="psx", bufs=3, space="PSUM") as psx:
                for ft in range(KT):
                    pt = psx.tile([128, C], BF16, tag="psxt")
                    for ct in range(CM):
                        nc.tensor.transpose(
                            pt[:, 128 * ct:128 * (ct + 1)],
                            Xg[:, ct, 128 * ft:128 * (ft + 1)], ident_bf[:])
                    nc.any.tensor_copy(XgT[:, ft, :], pt[:])

        hT = p_hT.tile([128, HM, C], BF16, tag="hT")
        with tc.tile_pool(name="ps1", bufs=3, space="PSUM") as ps1:
            for m in range(HM):
                ph = ps1.tile([128, C], F32, tag="ps1t")
                for k in range(KT):
                    nc.tensor.matmul(ph[:],
                                     w1_sb[:, k, 128 * m:128 * (m + 1)],
                                     XgT[:, k, :], start=(k == 0),
                                     stop=(k == KT - 1))
                nc.scalar.activation(hT[:, m, :], ph[:], AF.Gelu,
                                     bias=cvec["b1"][:, m:m + 1])

    yall = dram.tile([NCORES * C, D], BF16, addr_space="Shared")
    with (
        tc.tile_pool(name="p_y", bufs=1) as py,
        tc.tile_pool(name="ps4", bufs=1, space="PSUM") as ps4,
        tc.tile_pool(name="w2p", bufs=3) as w2p,
    ):
        y_sb = py.tile([128, CM, D], BF16, tag="y_sb")
        yps = {}
        for mc in range(CM):
            for nb in range(2):
                ypst = ps4.tile([128, 384], F32, tag=f"y{mc}{nb}")
                yps[(mc, nb)] = ypst
        for k in range(HM):
            w2t = w2p.tile([128, D], BF16, tag="w2t")
            nc.sync.dma_start(w2t[:],
                              ins["w2"][128 * k:128 * (k + 1), :])
            for mc in range(CM):
                for nb in range(2):
                    nc.tensor.matmul(
                        yps[(mc, nb)][:],
                        hT[:, k, 128 * mc:128 * (mc + 1)],
                        w2t[:, 384 * nb:384 * (nb + 1)],
                        start=(k == 0), stop=(k == HM - 1))
        for mc in range(CM):
            for nb in range(2):
                nc.any.tensor_copy(y_sb[:, mc, 384 * nb:384 * (nb + 1)],
                                   yps[(mc, nb)][:])
        nc.sync.dma_start(y_d[:].rearrange("(i p) d -> p i d", p=128),
                          y_sb[:])
        if "ysb" in dbg:
            nc.sync.dma_start(
                dbg["ysb"][:].rearrange("(i p) d -> p i d", p=128), y_sb[:])
    nc.gpsimd.collective_compute(
        "AllGather", OP.bypass, ins=[y_d.opt()], outs=[yall.opt()],
        replica_groups=[list(range(NCORES))])

    p_hT.release()

    # =====================================================================
    # Phase E: combine gather + weighted sum + output
    # =====================================================================
    with tc.tile_pool(name="p_e", bufs=1) as pe:
        g_tok = pe.tile([128, 4, D], BF16, tag="g_tok")
        nc.gpsimd.dma_gather(g_tok[:], yall[:], cidx128[:],
                             num_idxs=512, num_idxs_reg=512, elem_size=D)
        for i, sz in enumerate(NCH):
            dtile = sbw.tile([128, D], F32, tag="cmb_d")
            nc.vector.tensor_tensor(dtile[:sz, :], g_tok[:sz, i, :],
                                    g_tok[:sz, i + 2, :], op=OP.subtract)
            t2 = sbw.tile([128, D], F32, tag="cmb_t2")
            nc.vector.scalar_tensor_tensor(t2[:sz, :], dtile[:sz, :],
                                           g0w2[:sz, i:i + 1],
                                           g_tok[:sz, i + 2, :],
                                           op0=OP.mult, op1=OP.add)
            ocmb = sbw.tile([128, D], F32, tag="cmb_o")
            nc.vector.tensor_tensor(ocmb[:sz, :], t2[:sz, :],
                                    x1tok[:sz, i, :], op=OP.add)
            nc.sync.dma_start(out[128 * i:128 * i + sz, :], ocmb[:sz, :])

    # remaining debug taps
    if "gates" in dbg:
        for i in range(TW):
            nc.sync.dma_start(dbg["gates"][0:128, 8 * i:8 * (i + 1)],
                              gates[:, i, :])
    if "cidx" in dbg:
        nc.sync.dma_start(dbg["cidx"][:], cidx128[0:16, :])
    if "didx" in dbg:
        nc.sync.dma_start(dbg["didx"][:], didx128[0:16, :])
    if "g0w2" in dbg:
        nc.sync.dma_start(dbg["g0w2"][:], g0w2[:])
    if "x1T" in dbg:
        for k in range(KT):
            nc.sync.dma_start(dbg["x1T"][128 * k:128 * (k + 1), :],
                              x1T[:, k, :])

    sbw.release()
    sb.release()
    dram.release()


# ---------------------------------------------------------------------------
# host side
# ---------------------------------------------------------------------------

DBG_SPECS = {
    "xnT": (D, N), "gates": (128, TW * 8), "rank8": (8, TWPAD),
    "cidx": (16, 32), "didx": (16, C // 16), "g0w2": (128, 2),
    "ysb": (C, D), "x1T": (D, N),
}

_PROGRAM_CACHE = {}


def get_program(debug=False):
    key = bool(debug)
    if key in _PROGRAM_CACHE:
        return _PROGRAM_CACHE[key]
    nc = bacc.Bacc("TRN2", target_bir_lowering=False, debug=False,
                   enable_asserts=True, num_devices=NCORES)
    ins = {}

    def din(name, shape, dtype=F32):
        ins[name] = nc.dram_tensor(name, list(shape), dtype,
                                   kind="ExternalInput").ap()

    din("xT", (D, N))
    din("qkv_w", (D, 3 * D))
    din("qkv_b", (128, 18))
    din("proj_w", (D, D))
    din("proj_b", (128, KT))
    din("ln1_g", (128, KT))
    din("ln1_b", (128, KT))
    din("ln2_g", (128, KT))
    din("ln2_b", (128, KT))
    din("gate_w", (D, E))
    din("w1", (D, HID), BF16)
    din("b1", (128, HM))
    din("w2", (HID, D), BF16)
    din("b2", (1, D))
    din("tok0", (1, 1), I32)
    din("eoh", (128, 1))

    outs = {"out": nc.dram_tensor("out", [N, D], F32,
                                  kind="ExternalOutput").ap()}
    if debug:
        outs["dbg"] = {
            nm: nc.dram_tensor(f"dbg_{nm}", list(shp),
                               I16 if nm in ("cidx", "didx")
                               else (BF16 if nm == "ysb" else F32),
                               kind="ExternalOutput").ap()
            for nm, shp in DBG_SPECS.items()
        }

    with tile.TileContext(nc) as tc:
        build_block(tc, outs, ins)
    nc.compile()
    _PROGRAM_CACHE[key] = nc
    return nc


def to_bf16(x):
    import ml_dtypes
    return np.ascontiguousarray(x, np.float32).astype(ml_dtypes.bfloat16)


def round_f32r(x):
    """Round fp32 to fp32r (8e11m) with round-to-nearest-even."""
    b = np.ascontiguousarray(x, np.float32).view(np.uint32).copy()
    lsb = (b >> np.uint32(12)) & np.uint32(1)
    r = b + np.uint32(0x7FF) + lsb
    return (r & np.uint32(0xFFFFF000)).view(np.float32)


def prep_in_maps(inputs):
    f = {k: np.ascontiguousarray(np.asarray(v), dtype=np.float32)
         for k, v in inputs.items()}
    share = {
        "qkv_w": f["qkv_w"],
        "qkv_b": f["qkv_b"].reshape(18, 128).T.copy(),
        "proj_w": f["proj_w"],
        "proj_b": f["proj_b"].reshape(KT, 128).T.copy(),
        "ln1_g": f["ln1_g"].reshape(KT, 128).T.copy(),
        "ln1_b": f["ln1_b"].reshape(KT, 128).T.copy(),
        "ln2_g": f["ln2_g"].reshape(KT, 128).T.copy(),
        "ln2_b": f["ln2_b"].reshape(KT, 128).T.copy(),
        "gate_w": f["gate_w"],
    }
    in_maps = []
    for o in range(NCORES):
        m = dict(share)
        m["xT"] = f["x"][o].T.copy()
        m["w1"] = to_bf16(f["w1"][o])
        m["b1"] = f["b1"][o].reshape(HM, 128).T.copy()
        m["w2"] = to_bf16(f["w2"][o])
        m["b2"] = f["b2"][o].reshape(1, D).copy()
        m["tok0"] = np.array([[o * N]], np.int32)
        eoh = np.zeros((8, 1), np.float32)
        eoh[o, 0] = 1.0
        m["eoh"] = np.tile(eoh, (16, 1))
        in_maps.append(m)
    return in_maps


def kernel(**inputs):
    nc = get_program(debug=False)
    in_maps = prep_in_maps(inputs)
    res = bass_utils.run_bass_kernel_spmd(
        nc, in_maps, core_ids=list(range(NCORES)), trace=False)
    out = np.stack([r["out"] for r in res.results], axis=0)
    return out.astype(np.float32)
